# revision 35
# baseline (speedup 1.0000x reference)
"""Extended S5 SSM on 8 Trainium2 NeuronCores (Bass/Tile).

Sequence-parallel: L sharded across 8 cores (S=2048 each), feature-on-partition
layout. Complex diagonal scan via rotation factorization:
    x_k = lam*x_{k-1} + b_k,  lam = m*e^{i th}
    y_k = e^{-ik th} x_k  =>  y_k = m*y_{k-1} + e^{-ik th} b_k
One full-chunk real scan pair per core (T=S, no subchunk glue); cores chain
through one 8 KB AllGather of end-states per pass, with the homogeneous
correction applied afterwards.

  pass 1:  Bu = B @ u^T (PE, bf16), rotate, scan, un-rotate -> x1
  low rank: Ep = M @ shift(x1_corrected),  M = E @ Delta^T @ F  (host-fused)
  pass 2:  scan of (w1 + rot(Ep)), second AllGather
  out:     out^T = Cre xre2 - Cim xim2   (D*u added on host)
"""

import sys
from contextlib import ExitStack

import numpy as np

for _p in ("/opt/trn_rl_repo", "/root/.axon_site/_ro/trn_rl_repo"):
    if _p not in sys.path:
        sys.path.append(_p)

try:
    import ml_dtypes
except ImportError:
    ml_dtypes = None

# ---- problem geometry (hardcoded; harness contract) ----
L, H, P, R = 16384, 1024, 1024, 512
NCORES = 8

CFG_FULL = dict(L=16384)

_PROG_CACHE = {}

# packed small-table column layout (ptab)
_PT_COEFRE = 0      # 8 cols
_PT_COEFIM = 8      # 8 cols
_PT_LPR = 16        # Re(lam)
_PT_LPI = 17        # Im(lam)
_PT_COST1 = 18      # cos((S-1)th)
_PT_SINT1 = 19      # sin((S-1)th)
_PT_MVEC = 20       # m = |lam|
_PT_ZCOL = 21       # 0 on core0 else 1
_PT_NC = 22


def _emit(nc, tc, io, cfg):
    import concourse.mybir as mybir

    f32 = mybir.dt.float32
    bf16 = mybir.dt.bfloat16
    OP = mybir.AluOpType

    S = cfg["L"] // NCORES
    KH = H // 128
    PTP = P // 128
    HT = H // 128
    NM = S // 512

    V = nc.vector
    G = nc.gpsimd
    A = nc.scalar
    SP = nc.sync

    est = ExitStack()
    tabs = est.enter_context(tc.tile_pool(name="tabs", bufs=1))
    glue = est.enter_context(tc.tile_pool(name="glue", bufs=1))
    dram = est.enter_context(tc.tile_pool(name="dram", bufs=1, space="DRAM"))

    # ---------- DRAM scratch ----------
    wsp = [dram.tile([128, 2 * S], bf16, name=f"wsp{pt}", tag=f"wsp{pt}")
           for pt in range(PTP)]
    y2sp = [dram.tile([128, 2 * S], bf16, name=f"y2sp{pt}", tag=f"y2sp{pt}")
            for pt in range(PTP)]
    xe_in = [dram.tile([128, 2 * (P // 128)], f32, name=f"xe_in{e}",
                       tag=f"xe_in{e}")
             for e in range(2)]
    xe_out = [
        dram.tile([NCORES * 128, 2 * (P // 128)], f32, addr_space="Shared",
                  name=f"xe_out{e}", tag=f"xe_out{e}")
        for e in range(2)
    ]

    # ---------- small persistent state ----------
    gre_t = [[glue.tile([128, 1], f32, name=f"g{e}re{pt}", tag=f"g{e}re{pt}")
              for pt in range(PTP)] for e in range(2)]
    Gre_t = [[glue.tile([128, 1], f32, name=f"G{e}re{pt}", tag=f"G{e}re{pt}")
              for pt in range(PTP)] for e in range(2)]
    Gim_t = [[glue.tile([128, 1], f32, name=f"G{e}im{pt}", tag=f"G{e}im{pt}")
              for pt in range(PTP)] for e in range(2)]
    nGim_t = [glue.tile([128, 1], f32, name=f"nG0im{pt}", tag=f"nG0im{pt}")
              for pt in range(PTP)]

    ptab_t = []

    def exchange(exi, es_tile):
        """AllGather end states; per-pt carry scalars g and G = lam*g."""
        SP.dma_start(xe_in[exi][:], es_tile[:])
        G.collective_compute(
            "AllGather",
            mybir.AluOpType.bypass,
            replica_groups=[list(range(NCORES))],
            ins=[xe_in[exi].opt()],
            outs=[xe_out[exi].opt()],
        )
        xga = glue.tile([128, NCORES * PTP * 2], f32, name=f"xga{exi}",
                        tag=f"xga{exi}")
        SP.dma_start(
            xga.rearrange("p (r qc) -> p r qc", r=NCORES),
            xe_out[exi].rearrange("(r p) qc -> p r qc", p=128),
        )
        x3 = xga.rearrange("p (r q c) -> p r q c", r=NCORES, c=2)
        for pt in range(PTP):
            xer = x3[:, :, pt, 0]
            xei = x3[:, :, pt, 1]
            cr = ptab_t[pt][:, _PT_COEFRE : _PT_COEFRE + NCORES]
            ci = ptab_t[pt][:, _PT_COEFIM : _PT_COEFIM + NCORES]
            m1 = glue.tile([128, NCORES], f32, name="m1", tag="m1", bufs=2)
            m2 = glue.tile([128, NCORES], f32, name="m2", tag="m2", bufs=2)
            m3 = glue.tile([128, NCORES], f32, name="m3", tag="m3", bufs=2)
            V.tensor_tensor(m1[:], cr, xer, op=OP.mult)
            G.tensor_tensor(m2[:], ci, xei, op=OP.mult)
            V.tensor_tensor(m3[:], m1[:], m2[:], op=OP.subtract)
            V.tensor_reduce(gre_t[exi][pt][:], m3[:], axis=mybir.AxisListType.X,
                            op=OP.add)
            m4 = glue.tile([128, NCORES], f32, name="m4", tag="m4", bufs=2)
            m5 = glue.tile([128, NCORES], f32, name="m5", tag="m5", bufs=2)
            m6 = glue.tile([128, NCORES], f32, name="m6", tag="m6", bufs=2)
            G.tensor_tensor(m4[:], cr, xei, op=OP.mult)
            V.tensor_tensor(m5[:], ci, xer, op=OP.mult)
            G.tensor_tensor(m6[:], m4[:], m5[:], op=OP.add)
            gim = glue.tile([128, 1], f32, name="gim", tag="gim", bufs=2)
            V.tensor_reduce(gim[:], m6[:], axis=mybir.AxisListType.X, op=OP.add)
            lpr = ptab_t[pt][:, _PT_LPR : _PT_LPR + 1]
            lpi = ptab_t[pt][:, _PT_LPI : _PT_LPI + 1]
            ga = glue.tile([128, 1], f32, name="Ga", tag="Ga", bufs=2)
            gb = glue.tile([128, 1], f32, name="Gb", tag="Gb", bufs=2)
            V.tensor_scalar_mul(ga[:], gre_t[exi][pt][:], lpr)
            G.tensor_scalar_mul(gb[:], gim[:], lpi)
            V.tensor_tensor(Gre_t[exi][pt][:], ga[:], gb[:], op=OP.subtract)
            gc = glue.tile([128, 1], f32, name="Gc", tag="Gc", bufs=2)
            gd = glue.tile([128, 1], f32, name="Gd", tag="Gd", bufs=2)
            G.tensor_scalar_mul(gc[:], gim[:], lpr)
            V.tensor_scalar_mul(gd[:], gre_t[exi][pt][:], lpi)
            G.tensor_tensor(Gim_t[exi][pt][:], gc[:], gd[:], op=OP.add)
            if exi == 0:
                V.tensor_scalar_mul(nGim_t[pt][:], Gim_t[exi][pt][:], -1.0)

    def end_state(y_re_ap, y_im_ap, pt, sc_pool, es_tile):
        # x_end = e^{i (S-1) th} * y_last  -> pack (re,im) into es col block
        yr = y_re_ap[:, S - 1 : S]
        yi = y_im_ap[:, S - 1 : S]
        cT = ptab_t[pt][:, _PT_COST1 : _PT_COST1 + 1]
        sT = ptab_t[pt][:, _PT_SINT1 : _PT_SINT1 + 1]
        ea = sc_pool.tile([128, 1], f32, name="esa", tag="esa", bufs=2)
        eb = sc_pool.tile([128, 1], f32, name="esb", tag="esb", bufs=2)
        ec = sc_pool.tile([128, 1], f32, name="esc", tag="esc", bufs=2)
        ed = sc_pool.tile([128, 1], f32, name="esd", tag="esd", bufs=2)
        A.mul(ea[:], yr, cT)
        A.mul(eb[:], yi, sT)
        A.mul(ec[:], yr, sT)
        A.mul(ed[:], yi, cT)
        V.tensor_tensor(es_tile[:, 2 * pt : 2 * pt + 1], ea[:], eb[:],
                        op=OP.subtract)
        V.tensor_tensor(es_tile[:, 2 * pt + 1 : 2 * pt + 2], ec[:], ed[:],
                        op=OP.add)

    # ---------- startup loads (spread across queues) ----------
    es_x1 = ExitStack()
    x1r = es_x1.enter_context(tc.tile_pool(name="x1r", bufs=1))
    es_ub = ExitStack()
    utp = es_ub.enter_context(tc.tile_pool(name="utp", bufs=1))
    btp = es_ub.enter_context(tc.tile_pool(name="btp", bufs=1))

    ut_t, btre_t, btim_t = [], [], []
    for k in range(KH):
        t = utp.tile([128, S], bf16, name=f"ut{k}", tag=f"ut{k}")
        eng = SP if k < 4 else A
        eng.dma_start(t[:], io["uT"].ap()[k * 128 : (k + 1) * 128, :])
        ut_t.append(t)
    for k in range(KH):
        t = btp.tile([128, P], bf16, name=f"btre{k}", tag=f"btre{k}")
        SP.dma_start(t[:], io["BTre"].ap()[k * 128 : (k + 1) * 128, :])
        btre_t.append(t)
        t = btp.tile([128, P], bf16, name=f"btim{k}", tag=f"btim{k}")
        A.dma_start(t[:], io["BTim"].ap()[k * 128 : (k + 1) * 128, :])
        btim_t.append(t)
    ctab_t, stab_t = [], []
    for pt in range(PTP):
        r0 = pt * 128
        t = tabs.tile([128, S], bf16, name=f"ctab{pt}", tag=f"ctab{pt}")
        SP.dma_start(t[:], io["ctab"].ap()[r0 : r0 + 128, :])
        ctab_t.append(t)
        t = tabs.tile([128, S], bf16, name=f"stab{pt}", tag=f"stab{pt}")
        A.dma_start(t[:], io["stab"].ap()[r0 : r0 + 128, :])
        stab_t.append(t)
        t = tabs.tile([128, _PT_NC], f32, name=f"ptab{pt}", tag=f"ptab{pt}")
        G.dma_start(t[:], io["ptab"].ap()[r0 : r0 + 128, :])
        ptab_t.append(t)

    # ==============================================================
    # PHASE 1: Bu matmuls, rotation, full-chunk scans, end states
    # ==============================================================
    es1 = glue.tile([128, 2 * PTP], f32, name="es1", tag="es1")
    x1u_t = []
    with (
        tc.tile_pool(name="p1", bufs=2) as p1,
        tc.tile_pool(name="ps1", bufs=4, space="PSUM") as ps1,
    ):
        HW = S // 2
        for pt in range(PTP):
            pc = slice(pt * 128, (pt + 1) * 128)
            w = p1.tile([128, 2 * S], bf16, name="w", tag="w")
            wre = w[:, 0:S]
            wim = w[:, S : 2 * S]
            for h in range(2):
                hs = slice(h * HW, (h + 1) * HW)
                bur = p1.tile([128, HW], bf16, name="bur", tag="bur")
                bui = p1.tile([128, HW], bf16, name="bui", tag="bui")
                for n2 in range(2):
                    ns = slice(h * HW + n2 * 512, h * HW + (n2 + 1) * 512)
                    bs = slice(n2 * 512, (n2 + 1) * 512)
                    pre = ps1.tile([128, 512], f32, name="pre", tag="pre")
                    for k in range(KH):
                        nc.tensor.matmul(
                            pre[:], btre_t[k][:, pc], ut_t[k][:, ns],
                            start=(k == 0), stop=(k == KH - 1),
                        )
                    A.copy(bur[:, bs], pre[:])
                    pim = ps1.tile([128, 512], f32, name="pim", tag="pim")
                    for k in range(KH):
                        nc.tensor.matmul(
                            pim[:], btim_t[k][:, pc], ut_t[k][:, ns],
                            start=(k == 0), stop=(k == KH - 1),
                        )
                    A.copy(bui[:, bs], pim[:])
                # rotation: wre = c*bur + s*bui ; wim = c*bui - s*bur
                ct = ctab_t[pt][:, hs]
                st = stab_t[pt][:, hs]
                t1 = p1.tile([128, HW], bf16, name="t1", tag="t1", bufs=1)
                t2 = p1.tile([128, HW], bf16, name="t2", tag="t2", bufs=1)
                t3 = p1.tile([128, HW], bf16, name="t3", tag="t3", bufs=1)
                t4 = p1.tile([128, HW], bf16, name="t4", tag="t4", bufs=1)
                V.tensor_tensor(t1[:], bur[:], ct, op=OP.mult)
                G.tensor_tensor(t2[:], bui[:], st, op=OP.mult)
                G.tensor_tensor(t3[:], bui[:], ct, op=OP.mult)
                V.tensor_tensor(t4[:], bur[:], st, op=OP.mult)
                V.tensor_tensor(wre[:, hs], t1[:], t2[:], op=OP.add)
                G.tensor_tensor(wim[:, hs], t3[:], t4[:], op=OP.subtract)
            eng = SP if (pt % 2 == 0) else A
            eng.dma_start(wsp[pt][:], w[:])
            # full-chunk scans
            yre = p1.tile([128, S], bf16, name="yre", tag="yre", bufs=1)
            yim = p1.tile([128, S], bf16, name="yim", tag="yim", bufs=1)
            mb = ptab_t[pt][:, _PT_MVEC : _PT_MVEC + 1].broadcast_to((128, S))
            V.tensor_tensor_scan(yre[:], mb, wre, 0.0, op0=OP.mult, op1=OP.add)
            V.tensor_tensor_scan(yim[:], mb, wim, 0.0, op0=OP.mult, op1=OP.add)
            end_state(yre[:], yim[:], pt, p1, es1)
            # un-rotate: x1u = c*yre - s*yim (in halves to reuse t-slots)
            x1u = x1r.tile([128, S], bf16, name=f"x1u{pt}", tag=f"x1u{pt}")
            for h in range(2):
                hs = slice(h * HW, (h + 1) * HW)
                t5 = p1.tile([128, HW], bf16, name="t5", tag="t1", bufs=1)
                t6 = p1.tile([128, HW], bf16, name="t6", tag="t3", bufs=1)
                V.tensor_tensor(t5[:], yre[:, hs], ctab_t[pt][:, hs], op=OP.mult)
                G.tensor_tensor(t6[:], yim[:, hs], stab_t[pt][:, hs], op=OP.mult)
                G.tensor_tensor(x1u[:, hs], t5[:], t6[:], op=OP.subtract)
            x1u_t.append(x1u)

    es_ub.close()   # release uT, BT

    # ---------- carry exchange 1 ----------
    exchange(0, es1)

    # ==============================================================
    # PHASE 2: xsh build, Ep = M @ xsh, rot, scan 2
    # ==============================================================
    es2 = glue.tile([128, 2 * PTP], f32, name="es2", tag="es2")
    if True:
        es_p2 = ExitStack()
        mtp = es_p2.enter_context(tc.tile_pool(name="mtp", bufs=1))
        p2 = es_p2.enter_context(tc.tile_pool(name="p2", bufs=2))
        ps2 = es_p2.enter_context(tc.tile_pool(name="ps2", bufs=4, space="PSUM"))
        es_mcs = ExitStack()
        mcs = es_mcs.enter_context(tc.tile_pool(name="mcs", bufs=1))

        # xsh build, IN-PLACE into x1u (shifted add emitted before the col-0
        # overwrite); mc/ms/MT loads interleaved per pt
        mt_t = []
        xsh_t = x1u_t
        for pt in range(PTP):
            mc = mcs.tile([128, S], bf16, name=f"mc{pt}", tag="mc", bufs=2)
            SP.dma_start(mc[:], io["mctab"].ap()[pt * 128 : (pt + 1) * 128, :])
            ms = mcs.tile([128, S], bf16, name=f"ms{pt}", tag="ms", bufs=2)
            A.dma_start(ms[:], io["mstab"].ap()[pt * 128 : (pt + 1) * 128, :])
            t = mtp.tile([128, P], bf16, name=f"mt{pt}", tag=f"mt{pt}")
            A.dma_start(t[:], io["MT"].ap()[pt * 128 : (pt + 1) * 128, :])
            mt_t.append(t)
            x = x1u_t[pt]
            c1 = mcs.tile([128, S], bf16, name="c1", tag="c1", bufs=2)
            if pt % 2 == 0:
                A.mul(c1[:], mc[:], Gre_t[0][pt][:, 0:1])
            else:
                V.tensor_scalar_mul(c1[:], mc[:], Gre_t[0][pt][:, 0:1])
            s1 = mcs.tile([128, S], bf16, name="s1", tag="s1", bufs=1)
            G.tensor_scalar_mul(s1[:], ms[:], nGim_t[pt][:, 0:1])
            cs = mcs.tile([128, S], bf16, name="cs", tag="cs", bufs=1)
            V.tensor_tensor(cs[:], c1[:], s1[:], op=OP.add)
            V.tensor_tensor(x[:, 1:S], cs[:, 0 : S - 1],
                            x[:, 0 : S - 1], op=OP.add)
            V.tensor_copy(x[:, 0:1], gre_t[0][pt][:])

        es_mcs.close()  # release mctab/mstab

        for pt in range(PTP):
            pc = slice(pt * 128, (pt + 1) * 128)
            wl = p2.tile([128, 2 * S], bf16, name="wl", tag="wl", bufs=1)
            SP.dma_start(wl[:], wsp[pt][:])
            w2r = p2.tile([128, S], bf16, name="w2r", tag="w2r")
            w2i = p2.tile([128, S], bf16, name="w2i", tag="w2i")
            for n in range(NM):
                ns = slice(n * 512, (n + 1) * 512)
                epp = ps2.tile([128, 512], f32, name="epp", tag="epp")
                for k in range(PTP):
                    nc.tensor.matmul(
                        epp[:], mt_t[k][:, pc], xsh_t[k][:, ns],
                        start=(k == 0), stop=(k == PTP - 1),
                    )
                ep_sb = p2.tile([128, 512], bf16, name="ep_sb", tag="ep_sb",
                                bufs=2)
                A.copy(ep_sb[:], epp[:])
                ta = p2.tile([128, 512], bf16, name="ta", tag="ta", bufs=2)
                tb = p2.tile([128, 512], bf16, name="tb", tag="tb", bufs=2)
                G.tensor_tensor(ta[:], ctab_t[pt][:, ns], ep_sb[:], op=OP.mult)
                G.tensor_tensor(tb[:], stab_t[pt][:, ns], ep_sb[:], op=OP.mult)
                eng_a = V if (n % 2 == 0) else G
                eng_b = G if (n % 2 == 0) else V
                eng_a.tensor_tensor(w2r[:, ns], wl[:, ns], ta[:], op=OP.add)
                eng_b.tensor_tensor(
                    w2i[:, ns],
                    wl[:, S + n * 512 : S + (n + 1) * 512],
                    tb[:], op=OP.subtract,
                )
            zc = ptab_t[pt][:, _PT_ZCOL : _PT_ZCOL + 1]
            V.tensor_tensor(w2r[:, 0:1], w2r[:, 0:1], zc, op=OP.mult)
            G.tensor_tensor(w2i[:, 0:1], w2i[:, 0:1], zc, op=OP.mult)
            y2 = p2.tile([128, 2 * S], bf16, name="y2", tag="y2")
            y2r = y2[:, 0:S]
            y2i = y2[:, S : 2 * S]
            mb = ptab_t[pt][:, _PT_MVEC : _PT_MVEC + 1].broadcast_to((128, S))
            V.tensor_tensor_scan(y2r, mb, w2r[:], 0.0, op0=OP.mult, op1=OP.add)
            V.tensor_tensor_scan(y2i, mb, w2i[:], 0.0, op0=OP.mult, op1=OP.add)
            end_state(y2r, y2i, pt, p2, es2)
            A.dma_start(y2sp[pt][:], y2[:])

        es_p2.close()   # release MT, p2 working set
        es_x1.close()   # release x1 (consumed as xsh)

        # ---------- carry exchange 2 ----------
        exchange(1, es2)

        # ==========================================================
        # PHASE 3: y2 correction, rotate back, C projection (slice-wise)
        # ==========================================================
        with (
            tc.tile_pool(name="xrp", bufs=1) as xrp,
            tc.tile_pool(name="cpar", bufs=1) as cpar,
            tc.tile_pool(name="p3", bufs=2) as p3,
            tc.tile_pool(name="ps3", bufs=8, space="PSUM") as ps3,
        ):
            cre_t, nci_t = [], []
            for pt in range(PTP):
                t = cpar.tile([128, H], bf16, name=f"cre{pt}", tag=f"cre{pt}")
                SP.dma_start(t[:], io["CreT"].ap()[pt * 128 : (pt + 1) * 128, :])
                cre_t.append(t)
                t = cpar.tile([128, H], bf16, name=f"nci{pt}", tag=f"nci{pt}")
                A.dma_start(t[:], io["nCimT"].ap()[pt * 128 : (pt + 1) * 128, :])
                nci_t.append(t)
            xr_t = [xrp.tile([128, S], bf16, name=f"xr{pt}", tag=f"xr{pt}")
                    for pt in range(PTP)]
            xi_t = [xrp.tile([128, S], bf16, name=f"xi{pt}", tag=f"xi{pt}")
                    for pt in range(PTP)]
            for n in range(NM):
                ns = slice(n * 512, (n + 1) * 512)
                for pt in range(PTP):
                    y2l = p3.tile([128, 1024], bf16, name="y2l", tag="y2l",
                                  bufs=4)
                    eng = SP if (pt % 2 == 0) else A
                    eng.dma_start(
                        y2l.rearrange("p (c n) -> p c n", c=2),
                        y2sp[pt].rearrange("p (c s) -> p c s", c=2)[:, :, ns],
                    )
                    y2rl = y2l[:, 0:512]
                    y2il = y2l[:, 512:1024]
                    mpl = p3.tile([128, 512], bf16, name="mpl", tag="mpl",
                                  bufs=4)
                    eng2 = A if (pt % 2 == 0) else SP
                    eng2.dma_start(mpl[:],
                                   io["mptab"].ap()[pt * 128 : (pt + 1) * 128, ns])
                    mp1 = p3.tile([128, 512], bf16, name="mp1", tag="mp1")
                    mp2 = p3.tile([128, 512], bf16, name="mp2", tag="mp2")
                    V.tensor_scalar_mul(mp1[:], mpl[:], Gre_t[1][pt][:, 0:1])
                    V.tensor_scalar_mul(mp2[:], mpl[:], Gim_t[1][pt][:, 0:1])
                    yrc = p3.tile([128, 512], bf16, name="yrc", tag="yrc")
                    yic = p3.tile([128, 512], bf16, name="yic", tag="yic")
                    V.tensor_tensor(yrc[:], y2rl, mp1[:], op=OP.add)
                    G.tensor_tensor(yic[:], y2il, mp2[:], op=OP.add)
                    ua = p3.tile([128, 512], bf16, name="ua", tag="ua", bufs=1)
                    ub = p3.tile([128, 512], bf16, name="ub", tag="ub", bufs=1)
                    uc = p3.tile([128, 512], bf16, name="uc", tag="uc", bufs=1)
                    ud = p3.tile([128, 512], bf16, name="ud", tag="ud", bufs=1)
                    V.tensor_tensor(ua[:], ctab_t[pt][:, ns], yrc[:], op=OP.mult)
                    G.tensor_tensor(ub[:], stab_t[pt][:, ns], yic[:], op=OP.mult)
                    G.tensor_tensor(uc[:], stab_t[pt][:, ns], yrc[:], op=OP.mult)
                    V.tensor_tensor(ud[:], ctab_t[pt][:, ns], yic[:], op=OP.mult)
                    V.tensor_tensor(xr_t[pt][:, ns], ua[:], ub[:], op=OP.subtract)
                    G.tensor_tensor(xi_t[pt][:, ns], uc[:], ud[:], op=OP.add)
                for hb in range(HT):
                    mc_ = slice(hb * 128, (hb + 1) * 128)
                    op_ = ps3.tile([128, 512], f32, name="op", tag="op", bufs=8)
                    for k in range(PTP):
                        nc.tensor.matmul(
                            op_[:], cre_t[k][:, mc_],
                            xr_t[k][:, ns],
                            start=(k == 0), stop=False,
                        )
                    for k in range(PTP):
                        nc.tensor.matmul(
                            op_[:], nci_t[k][:, mc_],
                            xi_t[k][:, ns],
                            start=False, stop=(k == PTP - 1),
                        )
                    osb = p3.tile([128, 512], f32, name="osb", tag="osb", bufs=2)
                    A.copy(osb[:], op_[:])
                    SP.dma_start(io["outT"].ap()[mc_, ns], osb[:])

    est.close()


def build_program(cfg):
    import concourse.bacc as bacc
    import concourse.mybir as mybir
    import concourse.tile as tile

    f32 = mybir.dt.float32
    bf16 = mybir.dt.bfloat16
    S = cfg["L"] // NCORES

    nc = bacc.Bacc(
        "TRN2", target_bir_lowering=False, debug=False, num_devices=NCORES
    )
    io = {}
    ins = [
        ("uT", (H, S), bf16),
        ("BTre", (H, P), bf16), ("BTim", (H, P), bf16),
        ("CreT", (P, H), bf16), ("nCimT", (P, H), bf16),
        ("MT", (P, P), bf16),
        ("ctab", (P, S), bf16), ("stab", (P, S), bf16),
        ("mctab", (P, S), bf16), ("mstab", (P, S), bf16),
        ("mptab", (P, S), bf16),
        ("ptab", (P, _PT_NC), f32),
    ]
    for name, shape, dt_ in ins:
        io[name] = nc.dram_tensor(name, list(shape), dt_, kind="ExternalInput")
    io["outT"] = nc.dram_tensor("outT", [H, S], f32, kind="ExternalOutput")

    with tile.TileContext(nc) as tc:
        _emit(nc, tc, io, cfg)
    nc.compile()
    return nc


# ======================================================================
# host side
# ======================================================================

def make_tables(lam_re, lam_im, cfg):
    S = cfg["L"] // NCORES
    f32 = np.float32
    bf = ml_dtypes.bfloat16
    lam = lam_re.astype(np.float64) + 1j * lam_im.astype(np.float64)
    mag = np.abs(lam)
    th = np.angle(lam)
    k = np.arange(S)
    ang = np.outer(th, k)
    ctab = np.cos(ang)
    stab = np.sin(ang)
    with np.errstate(under="ignore"):
        mp = mag[:, None] ** k[None, :]
    tabs = dict(
        ctab=ctab.astype(bf), stab=stab.astype(bf),
        mctab=(mp * ctab).astype(bf), mstab=(mp * stab).astype(bf),
        mptab=mp.astype(bf),
    )
    coefre = np.zeros((NCORES, P, NCORES), f32)
    coefim = np.zeros((NCORES, P, NCORES), f32)
    for m in range(NCORES):
        for j in range(m):
            v = lam ** (S * (m - 1 - j))
            coefre[m, :, j] = np.real(v)
            coefim[m, :, j] = np.imag(v)
    ptab = np.zeros((NCORES, P, _PT_NC), f32)
    for m in range(NCORES):
        ptab[m, :, _PT_COEFRE : _PT_COEFRE + NCORES] = coefre[m]
        ptab[m, :, _PT_COEFIM : _PT_COEFIM + NCORES] = coefim[m]
        ptab[m, :, _PT_LPR] = np.real(lam)
        ptab[m, :, _PT_LPI] = np.imag(lam)
        ptab[m, :, _PT_COST1] = np.cos((S - 1) * th)
        ptab[m, :, _PT_SINT1] = np.sin((S - 1) * th)
        ptab[m, :, _PT_MVEC] = mag
        ptab[m, :, _PT_ZCOL] = 0.0 if m == 0 else 1.0
    return tabs, ptab


def make_in_maps(inputs, cfg):
    f32 = np.float32
    bf = ml_dtypes.bfloat16
    Lc = cfg["L"]
    S = Lc // NCORES
    u = np.ascontiguousarray(np.asarray(inputs["input_sequence"], f32)[:Lc])
    tabs, ptab = make_tables(
        np.asarray(inputs["Lambda_re"]), np.asarray(inputs["Lambda_im"]), cfg
    )
    M = (
        np.asarray(inputs["E"], np.float64)
        @ np.asarray(inputs["Delta"], np.float64).T
        @ np.asarray(inputs["F"], np.float64)
    )
    shared = dict(
        BTre=np.ascontiguousarray(np.asarray(inputs["B_re"], f32).T).astype(bf),
        BTim=np.ascontiguousarray(np.asarray(inputs["B_im"], f32).T).astype(bf),
        CreT=np.ascontiguousarray(np.asarray(inputs["C_re"], f32).T).astype(bf),
        nCimT=np.ascontiguousarray(-np.asarray(inputs["C_im"], f32).T).astype(bf),
        MT=np.ascontiguousarray(M.T).astype(bf),
        **tabs,
    )
    in_maps = []
    for m in range(NCORES):
        im = dict(shared)
        im["uT"] = np.ascontiguousarray(u[m * S : (m + 1) * S, :].T).astype(bf)
        im["ptab"] = np.ascontiguousarray(ptab[m])
        in_maps.append(im)
    return in_maps


def assemble_output(results, inputs, cfg):
    Lc = cfg["L"]
    S = Lc // NCORES
    out = np.empty((Lc, H), np.float32)
    for m in range(NCORES):
        out[m * S : (m + 1) * S, :] = results[m]["outT"].T
    # D*u term applied on host (elementwise on inputs; off the scan path)
    u = np.asarray(inputs["input_sequence"], np.float32)[:Lc]
    D = np.asarray(inputs["D"], np.float32)
    out += u * D
    out[0, :] = 0.0
    return out


def get_program(cfg_key="full"):
    if cfg_key not in _PROG_CACHE:
        _PROG_CACHE[cfg_key] = build_program(CFG_FULL)
    return _PROG_CACHE[cfg_key]


def run(inputs, trace=False, **kw):
    from concourse import bass_utils

    nc = get_program()
    in_maps = make_in_maps(inputs, CFG_FULL)
    res = bass_utils.run_bass_kernel_spmd(
        nc, in_maps, core_ids=list(range(NCORES)), trace=trace, **kw
    )
    return assemble_output(res.results, inputs, CFG_FULL), res


def kernel(**inputs):
    out, _ = run(inputs)
    return out


# revision 36
# speedup vs baseline: 1.0005x; 1.0005x over previous
"""Extended S5 SSM on 8 Trainium2 NeuronCores (Bass/Tile).

Sequence-parallel: L sharded across 8 cores (S=2048 each), feature-on-partition
layout. Complex diagonal scan via rotation factorization:
    x_k = lam*x_{k-1} + b_k,  lam = m*e^{i th}
    y_k = e^{-ik th} x_k  =>  y_k = m*y_{k-1} + e^{-ik th} b_k
One full-chunk real scan pair per core (T=S, no subchunk glue); cores chain
through one 8 KB AllGather of end-states per pass, with the homogeneous
correction applied afterwards.

  pass 1:  Bu = B @ u^T (PE, bf16), rotate, scan, un-rotate -> x1
  low rank: Ep = M @ shift(x1_corrected),  M = E @ Delta^T @ F  (host-fused)
  pass 2:  scan of (w1 + rot(Ep)), second AllGather
  out:     out^T = Cre xre2 - Cim xim2   (D*u added on host)
"""

import sys
from contextlib import ExitStack

import numpy as np

for _p in ("/opt/trn_rl_repo", "/root/.axon_site/_ro/trn_rl_repo"):
    if _p not in sys.path:
        sys.path.append(_p)

try:
    import ml_dtypes
except ImportError:
    ml_dtypes = None

# ---- problem geometry (hardcoded; harness contract) ----
L, H, P, R = 16384, 1024, 1024, 512
NCORES = 8

CFG_FULL = dict(L=16384)

_PROG_CACHE = {}

# packed small-table column layout (ptab)
_PT_COEFRE = 0      # 8 cols
_PT_COEFIM = 8      # 8 cols
_PT_LPR = 16        # Re(lam)
_PT_LPI = 17        # Im(lam)
_PT_COST1 = 18      # cos((S-1)th)
_PT_SINT1 = 19      # sin((S-1)th)
_PT_MVEC = 20       # m = |lam|
_PT_ZCOL = 21       # 0 on core0 else 1
_PT_NC = 22


def _emit(nc, tc, io, cfg):
    import concourse.mybir as mybir

    f32 = mybir.dt.float32
    bf16 = mybir.dt.bfloat16
    OP = mybir.AluOpType

    S = cfg["L"] // NCORES
    KH = H // 128
    PTP = P // 128
    HT = H // 128
    NM = S // 512

    V = nc.vector
    G = nc.gpsimd
    A = nc.scalar
    SP = nc.sync

    est = ExitStack()
    tabs = est.enter_context(tc.tile_pool(name="tabs", bufs=1))
    glue = est.enter_context(tc.tile_pool(name="glue", bufs=1))
    dram = est.enter_context(tc.tile_pool(name="dram", bufs=1, space="DRAM"))

    # ---------- DRAM scratch ----------
    wsp = [dram.tile([128, 2 * S], bf16, name=f"wsp{pt}", tag=f"wsp{pt}")
           for pt in range(PTP)]
    y2sp = [dram.tile([128, 2 * S], bf16, name=f"y2sp{pt}", tag=f"y2sp{pt}")
            for pt in range(PTP)]
    xe_in = [dram.tile([128, 2 * (P // 128)], f32, name=f"xe_in{e}",
                       tag=f"xe_in{e}")
             for e in range(2)]
    xe_out = [
        dram.tile([NCORES * 128, 2 * (P // 128)], f32, addr_space="Shared",
                  name=f"xe_out{e}", tag=f"xe_out{e}")
        for e in range(2)
    ]

    # ---------- small persistent state ----------
    gre_t = [[glue.tile([128, 1], f32, name=f"g{e}re{pt}", tag=f"g{e}re{pt}")
              for pt in range(PTP)] for e in range(2)]
    Gre_t = [[glue.tile([128, 1], f32, name=f"G{e}re{pt}", tag=f"G{e}re{pt}")
              for pt in range(PTP)] for e in range(2)]
    Gim_t = [[glue.tile([128, 1], f32, name=f"G{e}im{pt}", tag=f"G{e}im{pt}")
              for pt in range(PTP)] for e in range(2)]
    nGim_t = [glue.tile([128, 1], f32, name=f"nG0im{pt}", tag=f"nG0im{pt}")
              for pt in range(PTP)]

    ptab_t = []

    def exchange(exi, es_tile):
        """AllGather end states; per-pt carry scalars g and G = lam*g."""
        SP.dma_start(xe_in[exi][:], es_tile[:])
        G.collective_compute(
            "AllGather",
            mybir.AluOpType.bypass,
            replica_groups=[list(range(NCORES))],
            ins=[xe_in[exi].opt()],
            outs=[xe_out[exi].opt()],
        )
        xga = glue.tile([128, NCORES * PTP * 2], f32, name=f"xga{exi}",
                        tag=f"xga{exi}")
        SP.dma_start(
            xga.rearrange("p (r qc) -> p r qc", r=NCORES),
            xe_out[exi].rearrange("(r p) qc -> p r qc", p=128),
        )
        x3 = xga.rearrange("p (r q c) -> p r q c", r=NCORES, c=2)
        for pt in range(PTP):
            xer = x3[:, :, pt, 0]
            xei = x3[:, :, pt, 1]
            cr = ptab_t[pt][:, _PT_COEFRE : _PT_COEFRE + NCORES]
            ci = ptab_t[pt][:, _PT_COEFIM : _PT_COEFIM + NCORES]
            m1 = glue.tile([128, NCORES], f32, name="m1", tag="m1", bufs=2)
            m2 = glue.tile([128, NCORES], f32, name="m2", tag="m2", bufs=2)
            m3 = glue.tile([128, NCORES], f32, name="m3", tag="m3", bufs=2)
            V.tensor_tensor(m1[:], cr, xer, op=OP.mult)
            G.tensor_tensor(m2[:], ci, xei, op=OP.mult)
            V.tensor_tensor(m3[:], m1[:], m2[:], op=OP.subtract)
            V.tensor_reduce(gre_t[exi][pt][:], m3[:], axis=mybir.AxisListType.X,
                            op=OP.add)
            m4 = glue.tile([128, NCORES], f32, name="m4", tag="m4", bufs=2)
            m5 = glue.tile([128, NCORES], f32, name="m5", tag="m5", bufs=2)
            m6 = glue.tile([128, NCORES], f32, name="m6", tag="m6", bufs=2)
            G.tensor_tensor(m4[:], cr, xei, op=OP.mult)
            V.tensor_tensor(m5[:], ci, xer, op=OP.mult)
            G.tensor_tensor(m6[:], m4[:], m5[:], op=OP.add)
            gim = glue.tile([128, 1], f32, name="gim", tag="gim", bufs=2)
            V.tensor_reduce(gim[:], m6[:], axis=mybir.AxisListType.X, op=OP.add)
            lpr = ptab_t[pt][:, _PT_LPR : _PT_LPR + 1]
            lpi = ptab_t[pt][:, _PT_LPI : _PT_LPI + 1]
            ga = glue.tile([128, 1], f32, name="Ga", tag="Ga", bufs=2)
            gb = glue.tile([128, 1], f32, name="Gb", tag="Gb", bufs=2)
            V.tensor_scalar_mul(ga[:], gre_t[exi][pt][:], lpr)
            G.tensor_scalar_mul(gb[:], gim[:], lpi)
            V.tensor_tensor(Gre_t[exi][pt][:], ga[:], gb[:], op=OP.subtract)
            gc = glue.tile([128, 1], f32, name="Gc", tag="Gc", bufs=2)
            gd = glue.tile([128, 1], f32, name="Gd", tag="Gd", bufs=2)
            G.tensor_scalar_mul(gc[:], gim[:], lpr)
            V.tensor_scalar_mul(gd[:], gre_t[exi][pt][:], lpi)
            G.tensor_tensor(Gim_t[exi][pt][:], gc[:], gd[:], op=OP.add)
            if exi == 0:
                V.tensor_scalar_mul(nGim_t[pt][:], Gim_t[exi][pt][:], -1.0)

    def end_state(y_re_ap, y_im_ap, pt, sc_pool, es_tile):
        # x_end = e^{i (S-1) th} * y_last  -> pack (re,im) into es col block
        yr = y_re_ap[:, S - 1 : S]
        yi = y_im_ap[:, S - 1 : S]
        cT = ptab_t[pt][:, _PT_COST1 : _PT_COST1 + 1]
        sT = ptab_t[pt][:, _PT_SINT1 : _PT_SINT1 + 1]
        ea = sc_pool.tile([128, 1], f32, name="esa", tag="esa", bufs=2)
        eb = sc_pool.tile([128, 1], f32, name="esb", tag="esb", bufs=2)
        ec = sc_pool.tile([128, 1], f32, name="esc", tag="esc", bufs=2)
        ed = sc_pool.tile([128, 1], f32, name="esd", tag="esd", bufs=2)
        A.mul(ea[:], yr, cT)
        A.mul(eb[:], yi, sT)
        A.mul(ec[:], yr, sT)
        A.mul(ed[:], yi, cT)
        V.tensor_tensor(es_tile[:, 2 * pt : 2 * pt + 1], ea[:], eb[:],
                        op=OP.subtract)
        V.tensor_tensor(es_tile[:, 2 * pt + 1 : 2 * pt + 2], ec[:], ed[:],
                        op=OP.add)

    # ---------- startup loads (spread across queues) ----------
    es_x1 = ExitStack()
    x1r = es_x1.enter_context(tc.tile_pool(name="x1r", bufs=1))
    es_ub = ExitStack()
    utp = es_ub.enter_context(tc.tile_pool(name="utp", bufs=1))
    btp = es_ub.enter_context(tc.tile_pool(name="btp", bufs=1))

    ut_t, btre_t, btim_t = [], [], []
    for k in range(KH):
        t = utp.tile([128, S], bf16, name=f"ut{k}", tag=f"ut{k}")
        eng = SP if k < 4 else A
        eng.dma_start(t[:], io["uT"].ap()[k * 128 : (k + 1) * 128, :])
        ut_t.append(t)
    for k in range(KH):
        t = btp.tile([128, P], bf16, name=f"btre{k}", tag=f"btre{k}")
        SP.dma_start(t[:], io["BTre"].ap()[k * 128 : (k + 1) * 128, :])
        btre_t.append(t)
        t = btp.tile([128, P], bf16, name=f"btim{k}", tag=f"btim{k}")
        A.dma_start(t[:], io["BTim"].ap()[k * 128 : (k + 1) * 128, :])
        btim_t.append(t)
    ctab_t, stab_t = [], []
    for pt in range(PTP):
        r0 = pt * 128
        t = tabs.tile([128, S], bf16, name=f"ctab{pt}", tag=f"ctab{pt}")
        SP.dma_start(t[:], io["ctab"].ap()[r0 : r0 + 128, :])
        ctab_t.append(t)
        t = tabs.tile([128, S], bf16, name=f"stab{pt}", tag=f"stab{pt}")
        A.dma_start(t[:], io["stab"].ap()[r0 : r0 + 128, :])
        stab_t.append(t)
        t = tabs.tile([128, _PT_NC], f32, name=f"ptab{pt}", tag=f"ptab{pt}")
        G.dma_start(t[:], io["ptab"].ap()[r0 : r0 + 128, :])
        ptab_t.append(t)

    # ==============================================================
    # PHASE 1: Bu matmuls, rotation, full-chunk scans, end states
    # ==============================================================
    es1 = glue.tile([128, 2 * PTP], f32, name="es1", tag="es1")
    x1u_t = []
    with (
        tc.tile_pool(name="p1", bufs=2) as p1,
        tc.tile_pool(name="ps1", bufs=4, space="PSUM") as ps1,
    ):
        HW = S // 2
        for pt in range(PTP):
            pc = slice(pt * 128, (pt + 1) * 128)
            w = p1.tile([128, 2 * S], bf16, name="w", tag="w")
            wre = w[:, 0:S]
            wim = w[:, S : 2 * S]
            for h in range(2):
                hs = slice(h * HW, (h + 1) * HW)
                bur = p1.tile([128, HW], bf16, name="bur", tag="bur")
                bui = p1.tile([128, HW], bf16, name="bui", tag="bui")
                for n2 in range(2):
                    ns = slice(h * HW + n2 * 512, h * HW + (n2 + 1) * 512)
                    bs = slice(n2 * 512, (n2 + 1) * 512)
                    pre = ps1.tile([128, 512], f32, name="pre", tag="pre")
                    for k in range(KH):
                        nc.tensor.matmul(
                            pre[:], btre_t[k][:, pc], ut_t[k][:, ns],
                            start=(k == 0), stop=(k == KH - 1),
                        )
                    A.copy(bur[:, bs], pre[:])
                    pim = ps1.tile([128, 512], f32, name="pim", tag="pim")
                    for k in range(KH):
                        nc.tensor.matmul(
                            pim[:], btim_t[k][:, pc], ut_t[k][:, ns],
                            start=(k == 0), stop=(k == KH - 1),
                        )
                    A.copy(bui[:, bs], pim[:])
                # rotation: wre = c*bur + s*bui ; wim = c*bui - s*bur
                ct = ctab_t[pt][:, hs]
                st = stab_t[pt][:, hs]
                t1 = p1.tile([128, HW], bf16, name="t1", tag="t1", bufs=1)
                t2 = p1.tile([128, HW], bf16, name="t2", tag="t2", bufs=1)
                t3 = p1.tile([128, HW], bf16, name="t3", tag="t3", bufs=1)
                t4 = p1.tile([128, HW], bf16, name="t4", tag="t4", bufs=1)
                V.tensor_tensor(t1[:], bur[:], ct, op=OP.mult)
                G.tensor_tensor(t2[:], bui[:], st, op=OP.mult)
                G.tensor_tensor(t3[:], bui[:], ct, op=OP.mult)
                V.tensor_tensor(t4[:], bur[:], st, op=OP.mult)
                V.tensor_tensor(wre[:, hs], t1[:], t2[:], op=OP.add)
                G.tensor_tensor(wim[:, hs], t3[:], t4[:], op=OP.subtract)
            eng = SP if (pt % 2 == 0) else A
            eng.dma_start(wsp[pt][:], w[:])
            # full-chunk scans
            yre = p1.tile([128, S], bf16, name="yre", tag="yre", bufs=1)
            yim = p1.tile([128, S], bf16, name="yim", tag="yim", bufs=1)
            mb = ptab_t[pt][:, _PT_MVEC : _PT_MVEC + 1].broadcast_to((128, S))
            V.tensor_tensor_scan(yre[:], mb, wre, 0.0, op0=OP.mult, op1=OP.add)
            V.tensor_tensor_scan(yim[:], mb, wim, 0.0, op0=OP.mult, op1=OP.add)
            end_state(yre[:], yim[:], pt, p1, es1)
            # un-rotate: x1u = c*yre - s*yim (in halves to reuse t-slots)
            x1u = x1r.tile([128, S], bf16, name=f"x1u{pt}", tag=f"x1u{pt}")
            for h in range(2):
                hs = slice(h * HW, (h + 1) * HW)
                t5 = p1.tile([128, HW], bf16, name="t5", tag="t1", bufs=1)
                t6 = p1.tile([128, HW], bf16, name="t6", tag="t3", bufs=1)
                V.tensor_tensor(t5[:], yre[:, hs], ctab_t[pt][:, hs], op=OP.mult)
                G.tensor_tensor(t6[:], yim[:, hs], stab_t[pt][:, hs], op=OP.mult)
                G.tensor_tensor(x1u[:, hs], t5[:], t6[:], op=OP.subtract)
            x1u_t.append(x1u)

    es_ub.close()   # release uT, BT

    # ---------- carry exchange 1 ----------
    exchange(0, es1)

    # ==============================================================
    # PHASE 2: xsh build, Ep = M @ xsh, rot, scan 2
    # ==============================================================
    es2 = glue.tile([128, 2 * PTP], f32, name="es2", tag="es2")
    if True:
        es_p2 = ExitStack()
        mtp = es_p2.enter_context(tc.tile_pool(name="mtp", bufs=1))
        p2 = es_p2.enter_context(tc.tile_pool(name="p2", bufs=2))
        ps2 = es_p2.enter_context(tc.tile_pool(name="ps2", bufs=8, space="PSUM"))
        es_mcs = ExitStack()
        mcs = es_mcs.enter_context(tc.tile_pool(name="mcs", bufs=1))

        # xsh build, IN-PLACE into x1u (shifted add emitted before the col-0
        # overwrite); mc/ms/MT loads interleaved per pt
        mt_t = []
        xsh_t = x1u_t
        for pt in range(PTP):
            mc = mcs.tile([128, S], bf16, name=f"mc{pt}", tag="mc", bufs=3)
            SP.dma_start(mc[:], io["mctab"].ap()[pt * 128 : (pt + 1) * 128, :])
            ms = mcs.tile([128, S], bf16, name=f"ms{pt}", tag="ms", bufs=3)
            A.dma_start(ms[:], io["mstab"].ap()[pt * 128 : (pt + 1) * 128, :])
            t = mtp.tile([128, P], bf16, name=f"mt{pt}", tag=f"mt{pt}")
            A.dma_start(t[:], io["MT"].ap()[pt * 128 : (pt + 1) * 128, :])
            mt_t.append(t)
            x = x1u_t[pt]
            c1 = mcs.tile([128, S], bf16, name="c1", tag="c1", bufs=2)
            V.tensor_scalar_mul(c1[:], mc[:], Gre_t[0][pt][:, 0:1])
            s1 = mcs.tile([128, S], bf16, name="s1", tag="s1", bufs=1)
            G.tensor_scalar_mul(s1[:], ms[:], nGim_t[pt][:, 0:1])
            cs = mcs.tile([128, S], bf16, name="cs", tag="cs", bufs=1)
            V.tensor_tensor(cs[:], c1[:], s1[:], op=OP.add)
            V.tensor_tensor(x[:, 1:S], cs[:, 0 : S - 1],
                            x[:, 0 : S - 1], op=OP.add)
            V.tensor_copy(x[:, 0:1], gre_t[0][pt][:])

        es_mcs.close()  # release mctab/mstab

        for pt in range(PTP):
            pc = slice(pt * 128, (pt + 1) * 128)
            wl = p2.tile([128, 2 * S], bf16, name="wl", tag="wl", bufs=1)
            SP.dma_start(wl[:], wsp[pt][:])
            w2r = p2.tile([128, S], bf16, name="w2r", tag="w2r")
            w2i = p2.tile([128, S], bf16, name="w2i", tag="w2i")
            for n in range(NM):
                ns = slice(n * 512, (n + 1) * 512)
                epp = ps2.tile([128, 512], f32, name="epp", tag="epp")
                for k in range(PTP):
                    nc.tensor.matmul(
                        epp[:], mt_t[k][:, pc], xsh_t[k][:, ns],
                        start=(k == 0), stop=(k == PTP - 1),
                    )
                ep_sb = p2.tile([128, 512], bf16, name="ep_sb", tag="ep_sb",
                                bufs=2)
                A.copy(ep_sb[:], epp[:])
                ta = p2.tile([128, 512], bf16, name="ta", tag="ta", bufs=2)
                tb = p2.tile([128, 512], bf16, name="tb", tag="tb", bufs=2)
                G.tensor_tensor(ta[:], ctab_t[pt][:, ns], ep_sb[:], op=OP.mult)
                G.tensor_tensor(tb[:], stab_t[pt][:, ns], ep_sb[:], op=OP.mult)
                eng_a = V if (n % 2 == 0) else G
                eng_b = G if (n % 2 == 0) else V
                eng_a.tensor_tensor(w2r[:, ns], wl[:, ns], ta[:], op=OP.add)
                eng_b.tensor_tensor(
                    w2i[:, ns],
                    wl[:, S + n * 512 : S + (n + 1) * 512],
                    tb[:], op=OP.subtract,
                )
            zc = ptab_t[pt][:, _PT_ZCOL : _PT_ZCOL + 1]
            V.tensor_tensor(w2r[:, 0:1], w2r[:, 0:1], zc, op=OP.mult)
            G.tensor_tensor(w2i[:, 0:1], w2i[:, 0:1], zc, op=OP.mult)
            y2 = p2.tile([128, 2 * S], bf16, name="y2", tag="y2")
            y2r = y2[:, 0:S]
            y2i = y2[:, S : 2 * S]
            mb = ptab_t[pt][:, _PT_MVEC : _PT_MVEC + 1].broadcast_to((128, S))
            V.tensor_tensor_scan(y2r, mb, w2r[:], 0.0, op0=OP.mult, op1=OP.add)
            V.tensor_tensor_scan(y2i, mb, w2i[:], 0.0, op0=OP.mult, op1=OP.add)
            end_state(y2r, y2i, pt, p2, es2)
            A.dma_start(y2sp[pt][:], y2[:])

        es_p2.close()   # release MT, p2 working set
        es_x1.close()   # release x1 (consumed as xsh)

        # ---------- carry exchange 2 ----------
        exchange(1, es2)

        # ==========================================================
        # PHASE 3: y2 correction, rotate back, C projection (slice-wise)
        # ==========================================================
        with (
            tc.tile_pool(name="xrp", bufs=1) as xrp,
            tc.tile_pool(name="cpar", bufs=1) as cpar,
            tc.tile_pool(name="p3", bufs=2) as p3,
            tc.tile_pool(name="ps3", bufs=8, space="PSUM") as ps3,
        ):
            cre_t, nci_t = [], []
            for pt in range(PTP):
                t = cpar.tile([128, H], bf16, name=f"cre{pt}", tag=f"cre{pt}")
                SP.dma_start(t[:], io["CreT"].ap()[pt * 128 : (pt + 1) * 128, :])
                cre_t.append(t)
                t = cpar.tile([128, H], bf16, name=f"nci{pt}", tag=f"nci{pt}")
                A.dma_start(t[:], io["nCimT"].ap()[pt * 128 : (pt + 1) * 128, :])
                nci_t.append(t)
            xr_t = [xrp.tile([128, S], bf16, name=f"xr{pt}", tag=f"xr{pt}")
                    for pt in range(PTP)]
            xi_t = [xrp.tile([128, S], bf16, name=f"xi{pt}", tag=f"xi{pt}")
                    for pt in range(PTP)]
            for n in range(NM):
                ns = slice(n * 512, (n + 1) * 512)
                for pt in range(PTP):
                    y2l = p3.tile([128, 1024], bf16, name="y2l", tag="y2l",
                                  bufs=4)
                    eng = SP if (pt % 2 == 0) else A
                    eng.dma_start(
                        y2l.rearrange("p (c n) -> p c n", c=2),
                        y2sp[pt].rearrange("p (c s) -> p c s", c=2)[:, :, ns],
                    )
                    y2rl = y2l[:, 0:512]
                    y2il = y2l[:, 512:1024]
                    mpl = p3.tile([128, 512], bf16, name="mpl", tag="mpl",
                                  bufs=4)
                    eng2 = A if (pt % 2 == 0) else SP
                    eng2.dma_start(mpl[:],
                                   io["mptab"].ap()[pt * 128 : (pt + 1) * 128, ns])
                    mp1 = p3.tile([128, 512], bf16, name="mp1", tag="mp1")
                    mp2 = p3.tile([128, 512], bf16, name="mp2", tag="mp2")
                    V.tensor_scalar_mul(mp1[:], mpl[:], Gre_t[1][pt][:, 0:1])
                    V.tensor_scalar_mul(mp2[:], mpl[:], Gim_t[1][pt][:, 0:1])
                    yrc = p3.tile([128, 512], bf16, name="yrc", tag="yrc")
                    yic = p3.tile([128, 512], bf16, name="yic", tag="yic")
                    V.tensor_tensor(yrc[:], y2rl, mp1[:], op=OP.add)
                    G.tensor_tensor(yic[:], y2il, mp2[:], op=OP.add)
                    ua = p3.tile([128, 512], bf16, name="ua", tag="ua", bufs=1)
                    ub = p3.tile([128, 512], bf16, name="ub", tag="ub", bufs=1)
                    uc = p3.tile([128, 512], bf16, name="uc", tag="uc", bufs=1)
                    ud = p3.tile([128, 512], bf16, name="ud", tag="ud", bufs=1)
                    V.tensor_tensor(ua[:], ctab_t[pt][:, ns], yrc[:], op=OP.mult)
                    G.tensor_tensor(ub[:], stab_t[pt][:, ns], yic[:], op=OP.mult)
                    G.tensor_tensor(uc[:], stab_t[pt][:, ns], yrc[:], op=OP.mult)
                    V.tensor_tensor(ud[:], ctab_t[pt][:, ns], yic[:], op=OP.mult)
                    V.tensor_tensor(xr_t[pt][:, ns], ua[:], ub[:], op=OP.subtract)
                    G.tensor_tensor(xi_t[pt][:, ns], uc[:], ud[:], op=OP.add)
                for hb in range(HT):
                    mc_ = slice(hb * 128, (hb + 1) * 128)
                    op_ = ps3.tile([128, 512], f32, name="op", tag="op", bufs=8)
                    for k in range(PTP):
                        nc.tensor.matmul(
                            op_[:], cre_t[k][:, mc_],
                            xr_t[k][:, ns],
                            start=(k == 0), stop=False,
                        )
                    for k in range(PTP):
                        nc.tensor.matmul(
                            op_[:], nci_t[k][:, mc_],
                            xi_t[k][:, ns],
                            start=False, stop=(k == PTP - 1),
                        )
                    osb = p3.tile([128, 512], f32, name="osb", tag="osb", bufs=2)
                    A.copy(osb[:], op_[:])
                    SP.dma_start(io["outT"].ap()[mc_, ns], osb[:])

    est.close()


def build_program(cfg):
    import concourse.bacc as bacc
    import concourse.mybir as mybir
    import concourse.tile as tile

    f32 = mybir.dt.float32
    bf16 = mybir.dt.bfloat16
    S = cfg["L"] // NCORES

    nc = bacc.Bacc(
        "TRN2", target_bir_lowering=False, debug=False, num_devices=NCORES
    )
    io = {}
    ins = [
        ("uT", (H, S), bf16),
        ("BTre", (H, P), bf16), ("BTim", (H, P), bf16),
        ("CreT", (P, H), bf16), ("nCimT", (P, H), bf16),
        ("MT", (P, P), bf16),
        ("ctab", (P, S), bf16), ("stab", (P, S), bf16),
        ("mctab", (P, S), bf16), ("mstab", (P, S), bf16),
        ("mptab", (P, S), bf16),
        ("ptab", (P, _PT_NC), f32),
    ]
    for name, shape, dt_ in ins:
        io[name] = nc.dram_tensor(name, list(shape), dt_, kind="ExternalInput")
    io["outT"] = nc.dram_tensor("outT", [H, S], f32, kind="ExternalOutput")

    with tile.TileContext(nc) as tc:
        _emit(nc, tc, io, cfg)
    nc.compile()
    return nc


# ======================================================================
# host side
# ======================================================================

def make_tables(lam_re, lam_im, cfg):
    S = cfg["L"] // NCORES
    f32 = np.float32
    bf = ml_dtypes.bfloat16
    lam = lam_re.astype(np.float64) + 1j * lam_im.astype(np.float64)
    mag = np.abs(lam)
    th = np.angle(lam)
    k = np.arange(S)
    ang = np.outer(th, k)
    ctab = np.cos(ang)
    stab = np.sin(ang)
    with np.errstate(under="ignore"):
        mp = mag[:, None] ** k[None, :]
    tabs = dict(
        ctab=ctab.astype(bf), stab=stab.astype(bf),
        mctab=(mp * ctab).astype(bf), mstab=(mp * stab).astype(bf),
        mptab=mp.astype(bf),
    )
    coefre = np.zeros((NCORES, P, NCORES), f32)
    coefim = np.zeros((NCORES, P, NCORES), f32)
    for m in range(NCORES):
        for j in range(m):
            v = lam ** (S * (m - 1 - j))
            coefre[m, :, j] = np.real(v)
            coefim[m, :, j] = np.imag(v)
    ptab = np.zeros((NCORES, P, _PT_NC), f32)
    for m in range(NCORES):
        ptab[m, :, _PT_COEFRE : _PT_COEFRE + NCORES] = coefre[m]
        ptab[m, :, _PT_COEFIM : _PT_COEFIM + NCORES] = coefim[m]
        ptab[m, :, _PT_LPR] = np.real(lam)
        ptab[m, :, _PT_LPI] = np.imag(lam)
        ptab[m, :, _PT_COST1] = np.cos((S - 1) * th)
        ptab[m, :, _PT_SINT1] = np.sin((S - 1) * th)
        ptab[m, :, _PT_MVEC] = mag
        ptab[m, :, _PT_ZCOL] = 0.0 if m == 0 else 1.0
    return tabs, ptab


def make_in_maps(inputs, cfg):
    f32 = np.float32
    bf = ml_dtypes.bfloat16
    Lc = cfg["L"]
    S = Lc // NCORES
    u = np.ascontiguousarray(np.asarray(inputs["input_sequence"], f32)[:Lc])
    tabs, ptab = make_tables(
        np.asarray(inputs["Lambda_re"]), np.asarray(inputs["Lambda_im"]), cfg
    )
    M = (
        np.asarray(inputs["E"], np.float64)
        @ np.asarray(inputs["Delta"], np.float64).T
        @ np.asarray(inputs["F"], np.float64)
    )
    shared = dict(
        BTre=np.ascontiguousarray(np.asarray(inputs["B_re"], f32).T).astype(bf),
        BTim=np.ascontiguousarray(np.asarray(inputs["B_im"], f32).T).astype(bf),
        CreT=np.ascontiguousarray(np.asarray(inputs["C_re"], f32).T).astype(bf),
        nCimT=np.ascontiguousarray(-np.asarray(inputs["C_im"], f32).T).astype(bf),
        MT=np.ascontiguousarray(M.T).astype(bf),
        **tabs,
    )
    in_maps = []
    for m in range(NCORES):
        im = dict(shared)
        im["uT"] = np.ascontiguousarray(u[m * S : (m + 1) * S, :].T).astype(bf)
        im["ptab"] = np.ascontiguousarray(ptab[m])
        in_maps.append(im)
    return in_maps


def assemble_output(results, inputs, cfg):
    Lc = cfg["L"]
    S = Lc // NCORES
    out = np.empty((Lc, H), np.float32)
    for m in range(NCORES):
        out[m * S : (m + 1) * S, :] = results[m]["outT"].T
    # D*u term applied on host (elementwise on inputs; off the scan path)
    u = np.asarray(inputs["input_sequence"], np.float32)[:Lc]
    D = np.asarray(inputs["D"], np.float32)
    out += u * D
    out[0, :] = 0.0
    return out


def get_program(cfg_key="full"):
    if cfg_key not in _PROG_CACHE:
        _PROG_CACHE[cfg_key] = build_program(CFG_FULL)
    return _PROG_CACHE[cfg_key]


def run(inputs, trace=False, **kw):
    from concourse import bass_utils

    nc = get_program()
    in_maps = make_in_maps(inputs, CFG_FULL)
    res = bass_utils.run_bass_kernel_spmd(
        nc, in_maps, core_ids=list(range(NCORES)), trace=trace, **kw
    )
    return assemble_output(res.results, inputs, CFG_FULL), res


def kernel(**inputs):
    out, _ = run(inputs)
    return out


# revision 39
# speedup vs baseline: 1.0216x; 1.0211x over previous
"""Extended S5 SSM on 8 Trainium2 NeuronCores (Bass/Tile).

Sequence-parallel: L sharded across 8 cores (S=2048 each), feature-on-partition
layout. Complex diagonal scan via rotation factorization:
    x_k = lam*x_{k-1} + b_k,  lam = m*e^{i th}
    y_k = e^{-ik th} x_k  =>  y_k = m*y_{k-1} + e^{-ik th} b_k
One full-chunk real scan pair per core (T=S, no subchunk glue); cores chain
through one 8 KB AllGather of end-states per pass, with the homogeneous
correction applied afterwards.

  pass 1:  Bu = B @ u^T (PE, bf16), rotate, scan, un-rotate -> x1
  low rank: Ep = M @ shift(x1_corrected),  M = E @ Delta^T @ F  (host-fused)
  pass 2:  scan of (w1 + rot(Ep)), second AllGather
  out:     out^T = Cre xre2 - Cim xim2   (D*u added on host)
"""

import sys
from contextlib import ExitStack

import numpy as np

for _p in ("/opt/trn_rl_repo", "/root/.axon_site/_ro/trn_rl_repo"):
    if _p not in sys.path:
        sys.path.append(_p)

try:
    import ml_dtypes
except ImportError:
    ml_dtypes = None

# ---- problem geometry (hardcoded; harness contract) ----
L, H, P, R = 16384, 1024, 1024, 512
NCORES = 8

CFG_FULL = dict(L=16384)

_PROG_CACHE = {}

# packed small-table column layout (ptab)
_PT_COEFRE = 0      # 8 cols
_PT_COEFIM = 8      # 8 cols
_PT_LPR = 16        # Re(lam)
_PT_LPI = 17        # Im(lam)
_PT_COST1 = 18      # cos((S-1)th)
_PT_SINT1 = 19      # sin((S-1)th)
_PT_MVEC = 20       # m = |lam|
_PT_ZCOL = 21       # 0 on core0 else 1
_PT_NC = 22


def _emit(nc, tc, io, cfg):
    import concourse.mybir as mybir

    f32 = mybir.dt.float32
    bf16 = mybir.dt.bfloat16
    OP = mybir.AluOpType

    S = cfg["L"] // NCORES
    KH = H // 128
    PTP = P // 128
    HT = H // 128
    NM = S // 512

    V = nc.vector
    G = nc.gpsimd
    A = nc.scalar
    SP = nc.sync

    est = ExitStack()
    tabs = est.enter_context(tc.tile_pool(name="tabs", bufs=1))
    glue = est.enter_context(tc.tile_pool(name="glue", bufs=1))
    dram = est.enter_context(tc.tile_pool(name="dram", bufs=1, space="DRAM"))

    # ---------- DRAM scratch ----------
    wsp = [dram.tile([128, 2 * S], bf16, name=f"wsp{pt}", tag=f"wsp{pt}")
           for pt in range(PTP)]
    y2sp = [dram.tile([128, 2 * S], bf16, name=f"y2sp{pt}", tag=f"y2sp{pt}")
            for pt in range(PTP)]
    xe_in = [dram.tile([128, 2 * (P // 128)], f32, name=f"xe_in{e}",
                       tag=f"xe_in{e}")
             for e in range(2)]
    xe_out = [
        dram.tile([NCORES * 128, 2 * (P // 128)], f32, addr_space="Shared",
                  name=f"xe_out{e}", tag=f"xe_out{e}")
        for e in range(2)
    ]

    # ---------- small persistent state ----------
    gre_t = [[glue.tile([128, 1], f32, name=f"g{e}re{pt}", tag=f"g{e}re{pt}")
              for pt in range(PTP)] for e in range(2)]
    Gre_t = [[glue.tile([128, 1], f32, name=f"G{e}re{pt}", tag=f"G{e}re{pt}")
              for pt in range(PTP)] for e in range(2)]
    Gim_t = [[glue.tile([128, 1], f32, name=f"G{e}im{pt}", tag=f"G{e}im{pt}")
              for pt in range(PTP)] for e in range(2)]

    ptab_t = []

    def exchange(exi, es_tile):
        """AllGather end states; per-pt carry scalars g and G = lam*g."""
        SP.dma_start(xe_in[exi][:], es_tile[:])
        G.collective_compute(
            "AllGather",
            mybir.AluOpType.bypass,
            replica_groups=[list(range(NCORES))],
            ins=[xe_in[exi].opt()],
            outs=[xe_out[exi].opt()],
        )
        xga = glue.tile([128, NCORES * PTP * 2], f32, name=f"xga{exi}",
                        tag=f"xga{exi}")
        SP.dma_start(
            xga.rearrange("p (r qc) -> p r qc", r=NCORES),
            xe_out[exi].rearrange("(r p) qc -> p r qc", p=128),
        )
        x3 = xga.rearrange("p (r q c) -> p r q c", r=NCORES, c=2)
        for pt in range(PTP):
            xer = x3[:, :, pt, 0]
            xei = x3[:, :, pt, 1]
            cr = ptab_t[pt][:, _PT_COEFRE : _PT_COEFRE + NCORES]
            ci = ptab_t[pt][:, _PT_COEFIM : _PT_COEFIM + NCORES]
            m1 = glue.tile([128, NCORES], f32, name="m1", tag="m1", bufs=2)
            m2 = glue.tile([128, NCORES], f32, name="m2", tag="m2", bufs=2)
            m3 = glue.tile([128, NCORES], f32, name="m3", tag="m3", bufs=2)
            V.tensor_tensor(m1[:], cr, xer, op=OP.mult)
            G.tensor_tensor(m2[:], ci, xei, op=OP.mult)
            V.tensor_tensor(m3[:], m1[:], m2[:], op=OP.add)
            V.tensor_reduce(gre_t[exi][pt][:], m3[:], axis=mybir.AxisListType.X,
                            op=OP.add)
            m4 = glue.tile([128, NCORES], f32, name="m4", tag="m4", bufs=2)
            m5 = glue.tile([128, NCORES], f32, name="m5", tag="m5", bufs=2)
            m6 = glue.tile([128, NCORES], f32, name="m6", tag="m6", bufs=2)
            G.tensor_tensor(m4[:], cr, xei, op=OP.mult)
            V.tensor_tensor(m5[:], ci, xer, op=OP.mult)
            G.tensor_tensor(m6[:], m4[:], m5[:], op=OP.subtract)
            gim = glue.tile([128, 1], f32, name="gim", tag="gim", bufs=2)
            V.tensor_reduce(gim[:], m6[:], axis=mybir.AxisListType.X, op=OP.add)
            lpr = ptab_t[pt][:, _PT_LPR : _PT_LPR + 1]
            lpi = ptab_t[pt][:, _PT_LPI : _PT_LPI + 1]
            ga = glue.tile([128, 1], f32, name="Ga", tag="Ga", bufs=2)
            gb = glue.tile([128, 1], f32, name="Gb", tag="Gb", bufs=2)
            V.tensor_scalar_mul(ga[:], gre_t[exi][pt][:], lpr)
            G.tensor_scalar_mul(gb[:], gim[:], lpi)
            V.tensor_tensor(Gre_t[exi][pt][:], ga[:], gb[:], op=OP.add)
            gc = glue.tile([128, 1], f32, name="Gc", tag="Gc", bufs=2)
            gd = glue.tile([128, 1], f32, name="Gd", tag="Gd", bufs=2)
            G.tensor_scalar_mul(gc[:], gim[:], lpr)
            V.tensor_scalar_mul(gd[:], gre_t[exi][pt][:], lpi)
            G.tensor_tensor(Gim_t[exi][pt][:], gc[:], gd[:], op=OP.subtract)

    def end_state(y_re_ap, y_im_ap, pt, sc_pool, es_tile):
        # x_end = e^{i (S-1) th} * y_last  -> pack (re,im) into es col block
        yr = y_re_ap[:, S - 1 : S]
        yi = y_im_ap[:, S - 1 : S]
        cT = ptab_t[pt][:, _PT_COST1 : _PT_COST1 + 1]
        sT = ptab_t[pt][:, _PT_SINT1 : _PT_SINT1 + 1]
        ea = sc_pool.tile([128, 1], f32, name="esa", tag="esa", bufs=2)
        eb = sc_pool.tile([128, 1], f32, name="esb", tag="esb", bufs=2)
        ec = sc_pool.tile([128, 1], f32, name="esc", tag="esc", bufs=2)
        ed = sc_pool.tile([128, 1], f32, name="esd", tag="esd", bufs=2)
        A.mul(ea[:], yr, cT)
        A.mul(eb[:], yi, sT)
        A.mul(ec[:], yr, sT)
        A.mul(ed[:], yi, cT)
        V.tensor_tensor(es_tile[:, 2 * pt : 2 * pt + 1], ea[:], eb[:],
                        op=OP.add)
        V.tensor_tensor(es_tile[:, 2 * pt + 1 : 2 * pt + 2], ed[:], ec[:],
                        op=OP.subtract)

    # ---------- startup loads (spread across queues) ----------
    es_x1 = ExitStack()
    x1r = es_x1.enter_context(tc.tile_pool(name="x1r", bufs=1))
    es_ub = ExitStack()
    utp = es_ub.enter_context(tc.tile_pool(name="utp", bufs=1))
    btp = es_ub.enter_context(tc.tile_pool(name="btp", bufs=1))

    ut_t, btre_t, btim_t = [], [], []
    for k in range(KH):
        t = utp.tile([128, S], bf16, name=f"ut{k}", tag=f"ut{k}")
        eng = SP if k < 4 else A
        eng.dma_start(t[:], io["uT"].ap()[k * 128 : (k + 1) * 128, :])
        ut_t.append(t)
    for k in range(KH):
        t = btp.tile([128, P], bf16, name=f"btre{k}", tag=f"btre{k}")
        SP.dma_start(t[:], io["BTre"].ap()[k * 128 : (k + 1) * 128, :])
        btre_t.append(t)
        t = btp.tile([128, P], bf16, name=f"btim{k}", tag=f"btim{k}")
        A.dma_start(t[:], io["BTim"].ap()[k * 128 : (k + 1) * 128, :])
        btim_t.append(t)
    ctab_t, stab_t = [], []
    for pt in range(PTP):
        r0 = pt * 128
        t = tabs.tile([128, S], bf16, name=f"ctab{pt}", tag=f"ctab{pt}")
        SP.dma_start(t[:], io["ctab"].ap()[r0 : r0 + 128, :])
        ctab_t.append(t)
        t = tabs.tile([128, S], bf16, name=f"stab{pt}", tag=f"stab{pt}")
        A.dma_start(t[:], io["stab"].ap()[r0 : r0 + 128, :])
        stab_t.append(t)
        t = tabs.tile([128, _PT_NC], f32, name=f"ptab{pt}", tag=f"ptab{pt}")
        G.dma_start(t[:], io["ptab"].ap()[r0 : r0 + 128, :])
        ptab_t.append(t)

    # ==============================================================
    # PHASE 1: Bu matmuls, rotation, full-chunk scans, end states
    # ==============================================================
    es1 = glue.tile([128, 2 * PTP], f32, name="es1", tag="es1")
    x1u_t = []
    with (
        tc.tile_pool(name="p1", bufs=2) as p1,
        tc.tile_pool(name="ps1", bufs=4, space="PSUM") as ps1,
    ):
        HW = S // 2
        for pt in range(PTP):
            pc = slice(pt * 128, (pt + 1) * 128)
            w = p1.tile([128, 2 * S], bf16, name="w", tag="w")
            wre = w[:, 0:S]
            wim = w[:, S : 2 * S]
            for h in range(2):
                hs = slice(h * HW, (h + 1) * HW)
                bur = p1.tile([128, HW], bf16, name="bur", tag="bur")
                bui = p1.tile([128, HW], bf16, name="bui", tag="bui")
                for n2 in range(2):
                    ns = slice(h * HW + n2 * 512, h * HW + (n2 + 1) * 512)
                    bs = slice(n2 * 512, (n2 + 1) * 512)
                    pre = ps1.tile([128, 512], f32, name="pre", tag="pre")
                    for k in range(KH):
                        nc.tensor.matmul(
                            pre[:], btre_t[k][:, pc], ut_t[k][:, ns],
                            start=(k == 0), stop=(k == KH - 1),
                        )
                    A.copy(bur[:, bs], pre[:])
                    pim = ps1.tile([128, 512], f32, name="pim", tag="pim")
                    for k in range(KH):
                        nc.tensor.matmul(
                            pim[:], btim_t[k][:, pc], ut_t[k][:, ns],
                            start=(k == 0), stop=(k == KH - 1),
                        )
                    A.copy(bui[:, bs], pim[:])
                # rotation: wre = c*bur + s*bui ; wim = c*bui - s*bur
                ct = ctab_t[pt][:, hs]
                st = stab_t[pt][:, hs]
                t1 = p1.tile([128, HW], bf16, name="t1", tag="t1", bufs=1)
                t2 = p1.tile([128, HW], bf16, name="t2", tag="t2", bufs=1)
                t3 = p1.tile([128, HW], bf16, name="t3", tag="t3", bufs=1)
                t4 = p1.tile([128, HW], bf16, name="t4", tag="t4", bufs=1)
                V.tensor_tensor(t1[:], bur[:], ct, op=OP.mult)
                G.tensor_tensor(t2[:], bui[:], st, op=OP.mult)
                G.tensor_tensor(t3[:], bui[:], ct, op=OP.mult)
                V.tensor_tensor(t4[:], bur[:], st, op=OP.mult)
                V.tensor_tensor(wre[:, hs], t1[:], t2[:], op=OP.add)
                G.tensor_tensor(wim[:, hs], t4[:], t3[:], op=OP.subtract)
            eng = SP if (pt % 2 == 0) else A
            eng.dma_start(wsp[pt][:], w[:])
            # full-chunk scans
            yre = p1.tile([128, S], bf16, name="yre", tag="yre", bufs=1)
            yim = p1.tile([128, S], bf16, name="yim", tag="yim", bufs=1)
            mb = ptab_t[pt][:, _PT_MVEC : _PT_MVEC + 1].broadcast_to((128, S))
            V.tensor_tensor_scan(yre[:], mb, wre, 0.0, op0=OP.mult, op1=OP.add)
            V.tensor_tensor_scan(yim[:], mb, wim, 0.0, op0=OP.mult, op1=OP.add)
            end_state(yre[:], yim[:], pt, p1, es1)
            # un-rotate: x1u = c*yre - s*yim (in halves to reuse t-slots)
            x1u = x1r.tile([128, S], bf16, name=f"x1u{pt}", tag=f"x1u{pt}")
            for h in range(2):
                hs = slice(h * HW, (h + 1) * HW)
                t5 = p1.tile([128, HW], bf16, name="t5", tag="t1", bufs=1)
                t6 = p1.tile([128, HW], bf16, name="t6", tag="t3", bufs=1)
                V.tensor_tensor(t5[:], yre[:, hs], ctab_t[pt][:, hs], op=OP.mult)
                G.tensor_tensor(t6[:], yim[:, hs], stab_t[pt][:, hs], op=OP.mult)
                G.tensor_tensor(x1u[:, hs], t5[:], t6[:], op=OP.add)
            x1u_t.append(x1u)

    es_ub.close()   # release uT, BT

    # ---------- carry exchange 1 ----------
    exchange(0, es1)

    # ==============================================================
    # PHASE 2: xsh build, Ep = M @ xsh, rot, scan 2
    # ==============================================================
    es2 = glue.tile([128, 2 * PTP], f32, name="es2", tag="es2")
    if True:
        es_p2 = ExitStack()
        mtp = es_p2.enter_context(tc.tile_pool(name="mtp", bufs=1))
        p2 = es_p2.enter_context(tc.tile_pool(name="p2", bufs=2))
        ps2 = es_p2.enter_context(tc.tile_pool(name="ps2", bufs=8, space="PSUM"))
        es_mcs = ExitStack()
        mcs = es_mcs.enter_context(tc.tile_pool(name="mcs", bufs=1))

        # xsh build, IN-PLACE into x1u (shifted add emitted before the col-0
        # overwrite); mc/ms/MT loads interleaved per pt
        mt_t = []
        xsh_t = x1u_t
        for pt in range(PTP):
            mc = mcs.tile([128, S], bf16, name=f"mc{pt}", tag="mc", bufs=3)
            SP.dma_start(mc[:], io["mctab"].ap()[pt * 128 : (pt + 1) * 128, :])
            ms = mcs.tile([128, S], bf16, name=f"ms{pt}", tag="ms", bufs=3)
            A.dma_start(ms[:], io["mstab"].ap()[pt * 128 : (pt + 1) * 128, :])
            t = mtp.tile([128, P], bf16, name=f"mt{pt}", tag=f"mt{pt}")
            A.dma_start(t[:], io["MT"].ap()[pt * 128 : (pt + 1) * 128, :])
            mt_t.append(t)
            x = x1u_t[pt]
            c1 = mcs.tile([128, S], bf16, name="c1", tag="c1", bufs=2)
            V.tensor_scalar_mul(c1[:], mc[:], Gre_t[0][pt][:, 0:1])
            s1 = mcs.tile([128, S], bf16, name="s1", tag="s1", bufs=1)
            G.tensor_scalar_mul(s1[:], ms[:], Gim_t[0][pt][:, 0:1])
            cs = mcs.tile([128, S], bf16, name="cs", tag="cs", bufs=1)
            V.tensor_tensor(cs[:], c1[:], s1[:], op=OP.add)
            V.tensor_tensor(x[:, 1:S], cs[:, 0 : S - 1],
                            x[:, 0 : S - 1], op=OP.add)
            V.tensor_copy(x[:, 0:1], gre_t[0][pt][:])

        es_mcs.close()  # release mctab/mstab

        for pt in range(PTP):
            pc = slice(pt * 128, (pt + 1) * 128)
            w2r = p2.tile([128, S], bf16, name="w2r", tag="w2r")
            w2i = p2.tile([128, S], bf16, name="w2i", tag="w2i")
            for n in range(NM):
                ns = slice(n * 512, (n + 1) * 512)
                epp = ps2.tile([128, 512], f32, name="epp", tag="epp")
                for k in range(PTP):
                    nc.tensor.matmul(
                        epp[:], mt_t[k][:, pc], xsh_t[k][:, ns],
                        start=(k == 0), stop=(k == PTP - 1),
                    )
                ep_sb = p2.tile([128, 512], bf16, name="ep_sb", tag="ep_sb",
                                bufs=2)
                A.copy(ep_sb[:], epp[:])
                G.tensor_tensor(w2r[:, ns], ctab_t[pt][:, ns], ep_sb[:],
                                op=OP.mult)
                G.tensor_tensor(w2i[:, ns], stab_t[pt][:, ns], ep_sb[:],
                                op=OP.mult)
            # w2 += spilled w1 (re, negated im) via DMA-accumulate (SWDGE)
            G.dma_start(w2r[:], wsp[pt][:, 0:S], accum_op=OP.add)
            G.dma_start(w2i[:], wsp[pt][:, S : 2 * S], accum_op=OP.add)
            zc = ptab_t[pt][:, _PT_ZCOL : _PT_ZCOL + 1]
            V.tensor_tensor(w2r[:, 0:1], w2r[:, 0:1], zc, op=OP.mult)
            G.tensor_tensor(w2i[:, 0:1], w2i[:, 0:1], zc, op=OP.mult)
            y2 = p2.tile([128, 2 * S], bf16, name="y2", tag="y2")
            y2r = y2[:, 0:S]
            y2i = y2[:, S : 2 * S]
            mb = ptab_t[pt][:, _PT_MVEC : _PT_MVEC + 1].broadcast_to((128, S))
            V.tensor_tensor_scan(y2r, mb, w2r[:], 0.0, op0=OP.mult, op1=OP.add)
            V.tensor_tensor_scan(y2i, mb, w2i[:], 0.0, op0=OP.mult, op1=OP.add)
            end_state(y2r, y2i, pt, p2, es2)
            SP.dma_start(y2sp[pt][:], y2[:])

        es_p2.close()   # release MT, p2 working set
        es_x1.close()   # release x1 (consumed as xsh)

        # ---------- carry exchange 2 ----------
        exchange(1, es2)

        # ==========================================================
        # PHASE 3: y2 correction, rotate back, C projection (slice-wise)
        # ==========================================================
        with (
            tc.tile_pool(name="xrp", bufs=1) as xrp,
            tc.tile_pool(name="cpar", bufs=1) as cpar,
            tc.tile_pool(name="p3", bufs=2) as p3,
            tc.tile_pool(name="ps3", bufs=8, space="PSUM") as ps3,
        ):
            cre_t, nci_t = [], []
            for pt in range(PTP):
                t = cpar.tile([128, H], bf16, name=f"cre{pt}", tag=f"cre{pt}")
                SP.dma_start(t[:], io["CreT"].ap()[pt * 128 : (pt + 1) * 128, :])
                cre_t.append(t)
                t = cpar.tile([128, H], bf16, name=f"nci{pt}", tag=f"nci{pt}")
                A.dma_start(t[:], io["nCimT"].ap()[pt * 128 : (pt + 1) * 128, :])
                nci_t.append(t)
            xr_t = [xrp.tile([128, S], bf16, name=f"xr{pt}", tag=f"xr{pt}")
                    for pt in range(PTP)]
            xi_t = [xrp.tile([128, S], bf16, name=f"xi{pt}", tag=f"xi{pt}")
                    for pt in range(PTP)]
            for n in range(NM):
                ns = slice(n * 512, (n + 1) * 512)
                for pt in range(PTP):
                    mpl = p3.tile([128, 512], bf16, name="mpl", tag="mpl",
                                  bufs=4)
                    eng2 = A if (pt % 2 == 0) else SP
                    eng2.dma_start(mpl[:],
                                   io["mptab"].ap()[pt * 128 : (pt + 1) * 128, ns])
                    y2l = p3.tile([128, 1024], bf16, name="y2l", tag="y2l",
                                  bufs=4)
                    eng = SP if (pt % 2 == 0) else A
                    eng.dma_start(
                        y2l.rearrange("p (c n) -> p c n", c=2),
                        y2sp[pt].rearrange("p (c s) -> p c s", c=2)[:, :, ns],
                    )
                    mp1 = p3.tile([128, 512], bf16, name="mp1", tag="mp1")
                    mp2 = p3.tile([128, 512], bf16, name="mp2", tag="mp2")
                    V.tensor_scalar_mul(mp1[:], mpl[:], Gre_t[1][pt][:, 0:1])
                    V.tensor_scalar_mul(mp2[:], mpl[:], Gim_t[1][pt][:, 0:1])
                    yrc = p3.tile([128, 512], bf16, name="yrc", tag="yrc")
                    yic = p3.tile([128, 512], bf16, name="yic", tag="yic")
                    V.tensor_tensor(yrc[:], y2l[:, 0:512], mp1[:], op=OP.add)
                    G.tensor_tensor(yic[:], y2l[:, 512:1024], mp2[:], op=OP.add)
                    ua = p3.tile([128, 512], bf16, name="ua", tag="ua", bufs=1)
                    ub = p3.tile([128, 512], bf16, name="ub", tag="ub", bufs=1)
                    uc = p3.tile([128, 512], bf16, name="uc", tag="uc", bufs=1)
                    ud = p3.tile([128, 512], bf16, name="ud", tag="ud", bufs=1)
                    V.tensor_tensor(ua[:], ctab_t[pt][:, ns], yrc[:], op=OP.mult)
                    G.tensor_tensor(ub[:], stab_t[pt][:, ns], yic[:], op=OP.mult)
                    G.tensor_tensor(uc[:], stab_t[pt][:, ns], yrc[:], op=OP.mult)
                    V.tensor_tensor(ud[:], ctab_t[pt][:, ns], yic[:], op=OP.mult)
                    V.tensor_tensor(xr_t[pt][:, ns], ua[:], ub[:], op=OP.add)
                    G.tensor_tensor(xi_t[pt][:, ns], uc[:], ud[:], op=OP.subtract)
                for hb in range(HT):
                    mc_ = slice(hb * 128, (hb + 1) * 128)
                    op_ = ps3.tile([128, 512], f32, name="op", tag="op", bufs=8)
                    for k in range(PTP):
                        nc.tensor.matmul(
                            op_[:], cre_t[k][:, mc_],
                            xr_t[k][:, ns],
                            start=(k == 0), stop=False,
                        )
                    for k in range(PTP):
                        nc.tensor.matmul(
                            op_[:], nci_t[k][:, mc_],
                            xi_t[k][:, ns],
                            start=False, stop=(k == PTP - 1),
                        )
                    osb = p3.tile([128, 512], f32, name="osb", tag="osb", bufs=2)
                    A.copy(osb[:], op_[:])
                    SP.dma_start(io["outT"].ap()[mc_, ns], osb[:])

    est.close()


def build_program(cfg):
    import concourse.bacc as bacc
    import concourse.mybir as mybir
    import concourse.tile as tile

    f32 = mybir.dt.float32
    bf16 = mybir.dt.bfloat16
    S = cfg["L"] // NCORES

    nc = bacc.Bacc(
        "TRN2", target_bir_lowering=False, debug=False, num_devices=NCORES
    )
    io = {}
    ins = [
        ("uT", (H, S), bf16),
        ("BTre", (H, P), bf16), ("BTim", (H, P), bf16),
        ("CreT", (P, H), bf16), ("nCimT", (P, H), bf16),
        ("MT", (P, P), bf16),
        ("ctab", (P, S), bf16), ("stab", (P, S), bf16),
        ("mctab", (P, S), bf16), ("mstab", (P, S), bf16),
        ("mptab", (P, S), bf16),
        ("ptab", (P, _PT_NC), f32),
    ]
    for name, shape, dt_ in ins:
        io[name] = nc.dram_tensor(name, list(shape), dt_, kind="ExternalInput")
    io["outT"] = nc.dram_tensor("outT", [H, S], f32, kind="ExternalOutput")

    with tile.TileContext(nc) as tc:
        _emit(nc, tc, io, cfg)
    nc.compile()
    return nc


# ======================================================================
# host side
# ======================================================================

def make_tables(lam_re, lam_im, cfg):
    S = cfg["L"] // NCORES
    f32 = np.float32
    bf = ml_dtypes.bfloat16
    lam = lam_re.astype(np.float64) + 1j * lam_im.astype(np.float64)
    mag = np.abs(lam)
    th = np.angle(lam)
    k = np.arange(S)
    ang = np.outer(th, k)
    ctab = np.cos(ang)
    stab = np.sin(ang)
    with np.errstate(under="ignore"):
        mp = mag[:, None] ** k[None, :]
    tabs = dict(
        ctab=ctab.astype(bf), stab=stab.astype(bf),
        mctab=(mp * ctab).astype(bf), mstab=(mp * stab).astype(bf),
        mptab=mp.astype(bf),
    )
    coefre = np.zeros((NCORES, P, NCORES), f32)
    coefim = np.zeros((NCORES, P, NCORES), f32)
    for m in range(NCORES):
        for j in range(m):
            v = lam ** (S * (m - 1 - j))
            coefre[m, :, j] = np.real(v)
            coefim[m, :, j] = np.imag(v)
    ptab = np.zeros((NCORES, P, _PT_NC), f32)
    for m in range(NCORES):
        ptab[m, :, _PT_COEFRE : _PT_COEFRE + NCORES] = coefre[m]
        ptab[m, :, _PT_COEFIM : _PT_COEFIM + NCORES] = coefim[m]
        ptab[m, :, _PT_LPR] = np.real(lam)
        ptab[m, :, _PT_LPI] = np.imag(lam)
        ptab[m, :, _PT_COST1] = np.cos((S - 1) * th)
        ptab[m, :, _PT_SINT1] = np.sin((S - 1) * th)
        ptab[m, :, _PT_MVEC] = mag
        ptab[m, :, _PT_ZCOL] = 0.0 if m == 0 else 1.0
    return tabs, ptab


def make_in_maps(inputs, cfg):
    f32 = np.float32
    bf = ml_dtypes.bfloat16
    Lc = cfg["L"]
    S = Lc // NCORES
    u = np.ascontiguousarray(np.asarray(inputs["input_sequence"], f32)[:Lc])
    tabs, ptab = make_tables(
        np.asarray(inputs["Lambda_re"]), np.asarray(inputs["Lambda_im"]), cfg
    )
    M = (
        np.asarray(inputs["E"], np.float64)
        @ np.asarray(inputs["Delta"], np.float64).T
        @ np.asarray(inputs["F"], np.float64)
    )
    shared = dict(
        BTre=np.ascontiguousarray(np.asarray(inputs["B_re"], f32).T).astype(bf),
        BTim=np.ascontiguousarray(np.asarray(inputs["B_im"], f32).T).astype(bf),
        CreT=np.ascontiguousarray(np.asarray(inputs["C_re"], f32).T).astype(bf),
        nCimT=np.ascontiguousarray(-np.asarray(inputs["C_im"], f32).T).astype(bf),
        MT=np.ascontiguousarray(M.T).astype(bf),
        **tabs,
    )
    in_maps = []
    for m in range(NCORES):
        im = dict(shared)
        im["uT"] = np.ascontiguousarray(u[m * S : (m + 1) * S, :].T).astype(bf)
        im["ptab"] = np.ascontiguousarray(ptab[m])
        in_maps.append(im)
    return in_maps


def assemble_output(results, inputs, cfg):
    Lc = cfg["L"]
    S = Lc // NCORES
    out = np.empty((Lc, H), np.float32)
    for m in range(NCORES):
        out[m * S : (m + 1) * S, :] = results[m]["outT"].T
    # D*u term applied on host (elementwise on inputs; off the scan path)
    u = np.asarray(inputs["input_sequence"], np.float32)[:Lc]
    D = np.asarray(inputs["D"], np.float32)
    out += u * D
    out[0, :] = 0.0
    return out


def get_program(cfg_key="full"):
    if cfg_key not in _PROG_CACHE:
        _PROG_CACHE[cfg_key] = build_program(CFG_FULL)
    return _PROG_CACHE[cfg_key]


def run(inputs, trace=False, **kw):
    from concourse import bass_utils

    nc = get_program()
    in_maps = make_in_maps(inputs, CFG_FULL)
    res = bass_utils.run_bass_kernel_spmd(
        nc, in_maps, core_ids=list(range(NCORES)), trace=trace, **kw
    )
    return assemble_output(res.results, inputs, CFG_FULL), res


def kernel(**inputs):
    out, _ = run(inputs)
    return out


# revision 42
# speedup vs baseline: 1.0387x; 1.0167x over previous
"""Extended S5 SSM on 8 Trainium2 NeuronCores (Bass/Tile).

Sequence-parallel: L sharded across 8 cores (S=2048 each), feature-on-partition
layout. Complex diagonal scan via rotation factorization:
    x_k = lam*x_{k-1} + b_k,  lam = m*e^{i th}
    y_k = e^{-ik th} x_k  =>  y_k = m*y_{k-1} + e^{-ik th} b_k
One full-chunk real scan pair per core (T=S, no subchunk glue); cores chain
through one 8 KB AllGather of end-states per pass, with the homogeneous
correction applied afterwards.

  pass 1:  Bu = B @ u^T (PE, bf16), rotate, scan, un-rotate -> x1
  low rank: Ep = M @ shift(x1_corrected),  M = E @ Delta^T @ F  (host-fused)
  pass 2:  scan of (w1 + rot(Ep)), second AllGather
  out:     out^T = Cre xre2 - Cim xim2   (D*u added on host)
"""

import sys
from contextlib import ExitStack

import numpy as np

for _p in ("/opt/trn_rl_repo", "/root/.axon_site/_ro/trn_rl_repo"):
    if _p not in sys.path:
        sys.path.append(_p)

try:
    import ml_dtypes
except ImportError:
    ml_dtypes = None

# ---- problem geometry (hardcoded; harness contract) ----
L, H, P, R = 16384, 1024, 1024, 512
NCORES = 8

CFG_FULL = dict(L=16384)

_PROG_CACHE = {}

# packed small-table column layout (ptab)
_PT_COEFRE = 0      # 8 cols
_PT_COEFIM = 8      # 8 cols
_PT_LPR = 16        # Re(lam)
_PT_LPI = 17        # Im(lam)
_PT_COST1 = 18      # cos((S-1)th)
_PT_SINT1 = 19      # sin((S-1)th)
_PT_MVEC = 20       # m = |lam|
_PT_ZCOL = 21       # 0 on core0 else 1
_PT_NC = 22


def _emit(nc, tc, io, cfg):
    import concourse.mybir as mybir

    f32 = mybir.dt.float32
    bf16 = mybir.dt.bfloat16
    OP = mybir.AluOpType

    S = cfg["L"] // NCORES
    KH = H // 128
    PTP = P // 128
    HT = H // 128
    NM = S // 512

    V = nc.vector
    G = nc.gpsimd
    A = nc.scalar
    SP = nc.sync

    est = ExitStack()
    tabs = est.enter_context(tc.tile_pool(name="tabs", bufs=1))
    glue = est.enter_context(tc.tile_pool(name="glue", bufs=1))
    dram = est.enter_context(tc.tile_pool(name="dram", bufs=1, space="DRAM"))

    # ---------- DRAM scratch ----------
    wsp = [dram.tile([128, 2 * S], bf16, name=f"wsp{pt}", tag=f"wsp{pt}")
           for pt in range(PTP)]
    y2sp = [dram.tile([128, 2 * S], bf16, name=f"y2sp{pt}", tag=f"y2sp{pt}")
            for pt in range(PTP)]
    xe_in = [dram.tile([128, 2 * (P // 128)], f32, name=f"xe_in{e}",
                       tag=f"xe_in{e}")
             for e in range(2)]
    xe_out = [
        dram.tile([NCORES * 128, 2 * (P // 128)], f32, addr_space="Shared",
                  name=f"xe_out{e}", tag=f"xe_out{e}")
        for e in range(2)
    ]

    # ---------- small persistent state ----------
    gre_t = [[glue.tile([128, 1], f32, name=f"g{e}re{pt}", tag=f"g{e}re{pt}")
              for pt in range(PTP)] for e in range(2)]
    Gre_t = [[glue.tile([128, 1], f32, name=f"G{e}re{pt}", tag=f"G{e}re{pt}")
              for pt in range(PTP)] for e in range(2)]
    Gim_t = [[glue.tile([128, 1], f32, name=f"G{e}im{pt}", tag=f"G{e}im{pt}")
              for pt in range(PTP)] for e in range(2)]

    ptab_t = []

    def exchange(exi, es_tile):
        """AllGather end states; per-pt carry scalars g and G = lam*g."""
        SP.dma_start(xe_in[exi][:], es_tile[:])
        G.collective_compute(
            "AllGather",
            mybir.AluOpType.bypass,
            replica_groups=[list(range(NCORES))],
            ins=[xe_in[exi].opt()],
            outs=[xe_out[exi].opt()],
        )
        xga = glue.tile([128, NCORES * PTP * 2], f32, name=f"xga{exi}",
                        tag=f"xga{exi}")
        SP.dma_start(
            xga.rearrange("p (r qc) -> p r qc", r=NCORES),
            xe_out[exi].rearrange("(r p) qc -> p r qc", p=128),
        )
        x3 = xga.rearrange("p (r q c) -> p r q c", r=NCORES, c=2)
        for pt in range(PTP):
            xer = x3[:, :, pt, 0]
            xei = x3[:, :, pt, 1]
            cr = ptab_t[pt][:, _PT_COEFRE : _PT_COEFRE + NCORES]
            ci = ptab_t[pt][:, _PT_COEFIM : _PT_COEFIM + NCORES]
            m1 = glue.tile([128, NCORES], f32, name="m1", tag="m1", bufs=2)
            m2 = glue.tile([128, NCORES], f32, name="m2", tag="m2", bufs=2)
            m3 = glue.tile([128, NCORES], f32, name="m3", tag="m3", bufs=2)
            V.tensor_tensor(m1[:], cr, xer, op=OP.mult)
            G.tensor_tensor(m2[:], ci, xei, op=OP.mult)
            V.tensor_tensor(m3[:], m1[:], m2[:], op=OP.add)
            V.tensor_reduce(gre_t[exi][pt][:], m3[:], axis=mybir.AxisListType.X,
                            op=OP.add)
            m4 = glue.tile([128, NCORES], f32, name="m4", tag="m4", bufs=2)
            m5 = glue.tile([128, NCORES], f32, name="m5", tag="m5", bufs=2)
            m6 = glue.tile([128, NCORES], f32, name="m6", tag="m6", bufs=2)
            G.tensor_tensor(m4[:], cr, xei, op=OP.mult)
            V.tensor_tensor(m5[:], ci, xer, op=OP.mult)
            G.tensor_tensor(m6[:], m4[:], m5[:], op=OP.subtract)
            gim = glue.tile([128, 1], f32, name="gim", tag="gim", bufs=2)
            V.tensor_reduce(gim[:], m6[:], axis=mybir.AxisListType.X, op=OP.add)
            lpr = ptab_t[pt][:, _PT_LPR : _PT_LPR + 1]
            lpi = ptab_t[pt][:, _PT_LPI : _PT_LPI + 1]
            ga = glue.tile([128, 1], f32, name="Ga", tag="Ga", bufs=2)
            gb = glue.tile([128, 1], f32, name="Gb", tag="Gb", bufs=2)
            V.tensor_scalar_mul(ga[:], gre_t[exi][pt][:], lpr)
            G.tensor_scalar_mul(gb[:], gim[:], lpi)
            V.tensor_tensor(Gre_t[exi][pt][:], ga[:], gb[:], op=OP.add)
            gc = glue.tile([128, 1], f32, name="Gc", tag="Gc", bufs=2)
            gd = glue.tile([128, 1], f32, name="Gd", tag="Gd", bufs=2)
            G.tensor_scalar_mul(gc[:], gim[:], lpr)
            V.tensor_scalar_mul(gd[:], gre_t[exi][pt][:], lpi)
            G.tensor_tensor(Gim_t[exi][pt][:], gc[:], gd[:], op=OP.subtract)

    def end_state(y_re_ap, y_im_ap, pt, sc_pool, es_tile):
        # x_end = e^{i (S-1) th} * y_last  -> pack (re,im) into es col block
        yr = y_re_ap[:, S - 1 : S]
        yi = y_im_ap[:, S - 1 : S]
        cT = ptab_t[pt][:, _PT_COST1 : _PT_COST1 + 1]
        sT = ptab_t[pt][:, _PT_SINT1 : _PT_SINT1 + 1]
        ea = sc_pool.tile([128, 1], f32, name="esa", tag="esa", bufs=2)
        eb = sc_pool.tile([128, 1], f32, name="esb", tag="esb", bufs=2)
        ec = sc_pool.tile([128, 1], f32, name="esc", tag="esc", bufs=2)
        ed = sc_pool.tile([128, 1], f32, name="esd", tag="esd", bufs=2)
        A.mul(ea[:], yr, cT)
        A.mul(eb[:], yi, sT)
        A.mul(ec[:], yr, sT)
        A.mul(ed[:], yi, cT)
        V.tensor_tensor(es_tile[:, 2 * pt : 2 * pt + 1], ea[:], eb[:],
                        op=OP.add)
        V.tensor_tensor(es_tile[:, 2 * pt + 1 : 2 * pt + 2], ed[:], ec[:],
                        op=OP.subtract)

    # ---------- startup loads (spread across queues) ----------
    es_x1 = ExitStack()
    x1r = es_x1.enter_context(tc.tile_pool(name="x1r", bufs=1))
    es_ub = ExitStack()
    utp = es_ub.enter_context(tc.tile_pool(name="utp", bufs=1))
    btp = es_ub.enter_context(tc.tile_pool(name="btp", bufs=1))

    # PE needs (btre_k, ut_k) pairs ASAP; btre on SP (fast 790ns each), ut on
    # Pool SWDGE; rotation tables timed to arrive just before first use.
    ut_t = [utp.tile([128, S], bf16, name=f"ut{k}", tag=f"ut{k}")
            for k in range(KH)]
    btre_t, btim_t = [], []
    ctab_t, stab_t = [], []
    for k in range(KH):
        t = btp.tile([128, P], bf16, name=f"btre{k}", tag=f"btre{k}")
        eng = SP if k < 4 else A
        eng.dma_start(t[:], io["BTre"].ap()[k * 128 : (k + 1) * 128, :])
        btre_t.append(t)
    for k in range(KH):
        t = btp.tile([128, P], bf16, name=f"btim{k}", tag=f"btim{k}")
        eng = SP if k < 4 else A
        eng.dma_start(t[:], io["BTim"].ap()[k * 128 : (k + 1) * 128, :])
        btim_t.append(t)
    for k in range(6):
        G.dma_start(ut_t[k][:], io["uT"].ap()[k * 128 : (k + 1) * 128, :])
    for pt in range(2):
        r0 = pt * 128
        t = tabs.tile([128, S], bf16, name=f"ctab{pt}", tag=f"ctab{pt}")
        SP.dma_start(t[:], io["ctab"].ap()[r0 : r0 + 128, :])
        ctab_t.append(t)
        t = tabs.tile([128, S], bf16, name=f"stab{pt}", tag=f"stab{pt}")
        A.dma_start(t[:], io["stab"].ap()[r0 : r0 + 128, :])
        stab_t.append(t)
        t = tabs.tile([128, _PT_NC], f32, name=f"ptab{pt}", tag=f"ptab{pt}")
        G.dma_start(t[:], io["ptab"].ap()[r0 : r0 + 128, :])
        ptab_t.append(t)
    for k in range(6, KH):
        G.dma_start(ut_t[k][:], io["uT"].ap()[k * 128 : (k + 1) * 128, :])
    for pt in range(2, PTP):
        r0 = pt * 128
        t = tabs.tile([128, S], bf16, name=f"ctab{pt}", tag=f"ctab{pt}")
        SP.dma_start(t[:], io["ctab"].ap()[r0 : r0 + 128, :])
        ctab_t.append(t)
        t = tabs.tile([128, S], bf16, name=f"stab{pt}", tag=f"stab{pt}")
        A.dma_start(t[:], io["stab"].ap()[r0 : r0 + 128, :])
        stab_t.append(t)
        t = tabs.tile([128, _PT_NC], f32, name=f"ptab{pt}", tag=f"ptab{pt}")
        G.dma_start(t[:], io["ptab"].ap()[r0 : r0 + 128, :])
        ptab_t.append(t)

    # ==============================================================
    # PHASE 1: Bu matmuls, rotation, full-chunk scans, end states
    # ==============================================================
    es1 = glue.tile([128, 2 * PTP], f32, name="es1", tag="es1")
    x1u_t = []
    with (
        tc.tile_pool(name="p1", bufs=2) as p1,
        tc.tile_pool(name="ps1", bufs=4, space="PSUM") as ps1,
    ):
        HW = S // 2
        for pt in range(PTP):
            pc = slice(pt * 128, (pt + 1) * 128)
            w = p1.tile([128, 2 * S], bf16, name="w", tag="w")
            wre = w[:, 0:S]
            wim = w[:, S : 2 * S]
            for h in range(2):
                hs = slice(h * HW, (h + 1) * HW)
                bur = p1.tile([128, HW], bf16, name="bur", tag="bur")
                bui = p1.tile([128, HW], bf16, name="bui", tag="bui")
                for n2 in range(2):
                    ns = slice(h * HW + n2 * 512, h * HW + (n2 + 1) * 512)
                    bs = slice(n2 * 512, (n2 + 1) * 512)
                    pre = ps1.tile([128, 512], f32, name="pre", tag="pre")
                    for k in range(KH):
                        nc.tensor.matmul(
                            pre[:], btre_t[k][:, pc], ut_t[k][:, ns],
                            start=(k == 0), stop=(k == KH - 1),
                        )
                    A.copy(bur[:, bs], pre[:])
                    pim = ps1.tile([128, 512], f32, name="pim", tag="pim")
                    for k in range(KH):
                        nc.tensor.matmul(
                            pim[:], btim_t[k][:, pc], ut_t[k][:, ns],
                            start=(k == 0), stop=(k == KH - 1),
                        )
                    A.copy(bui[:, bs], pim[:])
                # rotation: wre = c*bur + s*bui ; wim = c*bui - s*bur
                ct = ctab_t[pt][:, hs]
                st = stab_t[pt][:, hs]
                t1 = p1.tile([128, HW], bf16, name="t1", tag="t1", bufs=1)
                t2 = p1.tile([128, HW], bf16, name="t2", tag="t2", bufs=1)
                t3 = p1.tile([128, HW], bf16, name="t3", tag="t3", bufs=1)
                t4 = p1.tile([128, HW], bf16, name="t4", tag="t4", bufs=1)
                V.tensor_tensor(t1[:], bur[:], ct, op=OP.mult)
                G.tensor_tensor(t2[:], bui[:], st, op=OP.mult)
                G.tensor_tensor(t3[:], bui[:], ct, op=OP.mult)
                V.tensor_tensor(t4[:], bur[:], st, op=OP.mult)
                V.tensor_tensor(wre[:, hs], t1[:], t2[:], op=OP.add)
                G.tensor_tensor(wim[:, hs], t4[:], t3[:], op=OP.subtract)
            eng = SP if (pt % 2 == 0) else A
            eng.dma_start(wsp[pt][:], w[:])
            # full-chunk scans
            yre = p1.tile([128, S], bf16, name="yre", tag="yre", bufs=1)
            yim = p1.tile([128, S], bf16, name="yim", tag="yim", bufs=1)
            mb = ptab_t[pt][:, _PT_MVEC : _PT_MVEC + 1].broadcast_to((128, S))
            V.tensor_tensor_scan(yre[:], mb, wre, 0.0, op0=OP.mult, op1=OP.add)
            V.tensor_tensor_scan(yim[:], mb, wim, 0.0, op0=OP.mult, op1=OP.add)
            end_state(yre[:], yim[:], pt, p1, es1)
            # un-rotate: x1u = c*yre - s*yim (in halves to reuse t-slots)
            x1u = x1r.tile([128, S], bf16, name=f"x1u{pt}", tag=f"x1u{pt}")
            for h in range(2):
                hs = slice(h * HW, (h + 1) * HW)
                t5 = p1.tile([128, HW], bf16, name="t5", tag="t1", bufs=1)
                t6 = p1.tile([128, HW], bf16, name="t6", tag="t3", bufs=1)
                V.tensor_tensor(t5[:], yre[:, hs], ctab_t[pt][:, hs], op=OP.mult)
                G.tensor_tensor(t6[:], yim[:, hs], stab_t[pt][:, hs], op=OP.mult)
                G.tensor_tensor(x1u[:, hs], t5[:], t6[:], op=OP.add)
            x1u_t.append(x1u)

    es_ub.close()   # release uT, BT

    # ---------- carry exchange 1 ----------
    exchange(0, es1)

    # ==============================================================
    # PHASE 2: xsh build, Ep = M @ xsh, rot, scan 2
    # ==============================================================
    es2 = glue.tile([128, 2 * PTP], f32, name="es2", tag="es2")
    if True:
        es_p2 = ExitStack()
        mtp = es_p2.enter_context(tc.tile_pool(name="mtp", bufs=1))
        p2 = es_p2.enter_context(tc.tile_pool(name="p2", bufs=2))
        ps2 = es_p2.enter_context(tc.tile_pool(name="ps2", bufs=8, space="PSUM"))
        es_mcs = ExitStack()
        mcs = es_mcs.enter_context(tc.tile_pool(name="mcs", bufs=1))

        # xsh build, IN-PLACE into x1u (shifted add emitted before the col-0
        # overwrite); mc/ms/MT loads interleaved per pt
        mt_t = []
        xsh_t = x1u_t
        for pt in range(PTP):
            mc = mcs.tile([128, S], bf16, name=f"mc{pt}", tag="mc", bufs=3)
            SP.dma_start(mc[:], io["mctab"].ap()[pt * 128 : (pt + 1) * 128, :])
            ms = mcs.tile([128, S], bf16, name=f"ms{pt}", tag="ms", bufs=3)
            A.dma_start(ms[:], io["mstab"].ap()[pt * 128 : (pt + 1) * 128, :])
            t = mtp.tile([128, P], bf16, name=f"mt{pt}", tag=f"mt{pt}")
            A.dma_start(t[:], io["MT"].ap()[pt * 128 : (pt + 1) * 128, :])
            mt_t.append(t)
            x = x1u_t[pt]
            c1 = mcs.tile([128, S], bf16, name="c1", tag="c1", bufs=2)
            V.tensor_scalar_mul(c1[:], mc[:], Gre_t[0][pt][:, 0:1])
            s1 = mcs.tile([128, S], bf16, name="s1", tag="s1", bufs=1)
            G.tensor_scalar_mul(s1[:], ms[:], Gim_t[0][pt][:, 0:1])
            cs = mcs.tile([128, S], bf16, name="cs", tag="cs", bufs=1)
            V.tensor_tensor(cs[:], c1[:], s1[:], op=OP.add)
            V.tensor_tensor(x[:, 1:S], cs[:, 0 : S - 1],
                            x[:, 0 : S - 1], op=OP.add)
            V.tensor_copy(x[:, 0:1], gre_t[0][pt][:])

        es_mcs.close()  # release mctab/mstab

        for pt in range(PTP):
            pc = slice(pt * 128, (pt + 1) * 128)
            w2r = p2.tile([128, S], bf16, name="w2r", tag="w2r")
            w2i = p2.tile([128, S], bf16, name="w2i", tag="w2i")
            for n in range(NM):
                ns = slice(n * 512, (n + 1) * 512)
                epp = ps2.tile([128, 512], f32, name="epp", tag="epp")
                for k in range(PTP):
                    nc.tensor.matmul(
                        epp[:], mt_t[k][:, pc], xsh_t[k][:, ns],
                        start=(k == 0), stop=(k == PTP - 1),
                    )
                ep_sb = p2.tile([128, 512], bf16, name="ep_sb", tag="ep_sb",
                                bufs=2)
                A.copy(ep_sb[:], epp[:])
                G.tensor_tensor(w2r[:, ns], ctab_t[pt][:, ns], ep_sb[:],
                                op=OP.mult)
                G.tensor_tensor(w2i[:, ns], stab_t[pt][:, ns], ep_sb[:],
                                op=OP.mult)
            # w2 += spilled w1 (re, negated im) via DMA-accumulate (SWDGE)
            G.dma_start(w2r[:], wsp[pt][:, 0:S], accum_op=OP.add)
            G.dma_start(w2i[:], wsp[pt][:, S : 2 * S], accum_op=OP.add)
            zc = ptab_t[pt][:, _PT_ZCOL : _PT_ZCOL + 1]
            V.tensor_tensor(w2r[:, 0:1], w2r[:, 0:1], zc, op=OP.mult)
            G.tensor_tensor(w2i[:, 0:1], w2i[:, 0:1], zc, op=OP.mult)
            y2 = p2.tile([128, 2 * S], bf16, name="y2", tag="y2")
            y2r = y2[:, 0:S]
            y2i = y2[:, S : 2 * S]
            mb = ptab_t[pt][:, _PT_MVEC : _PT_MVEC + 1].broadcast_to((128, S))
            V.tensor_tensor_scan(y2r, mb, w2r[:], 0.0, op0=OP.mult, op1=OP.add)
            V.tensor_tensor_scan(y2i, mb, w2i[:], 0.0, op0=OP.mult, op1=OP.add)
            end_state(y2r, y2i, pt, p2, es2)
            SP.dma_start(y2sp[pt][:], y2[:])

        es_p2.close()   # release MT, p2 working set
        es_x1.close()   # release x1 (consumed as xsh)

        # ---------- carry exchange 2 ----------
        exchange(1, es2)

        # ==========================================================
        # PHASE 3: y2 correction, rotate back, C projection (slice-wise)
        # ==========================================================
        with (
            tc.tile_pool(name="xrp", bufs=1) as xrp,
            tc.tile_pool(name="cpar", bufs=1) as cpar,
            tc.tile_pool(name="p3", bufs=2) as p3,
            tc.tile_pool(name="ps3", bufs=8, space="PSUM") as ps3,
        ):
            cre_t, nci_t = [], []
            for pt in range(PTP):
                t = cpar.tile([128, H], bf16, name=f"cre{pt}", tag=f"cre{pt}")
                SP.dma_start(t[:], io["CreT"].ap()[pt * 128 : (pt + 1) * 128, :])
                cre_t.append(t)
                t = cpar.tile([128, H], bf16, name=f"nci{pt}", tag=f"nci{pt}")
                A.dma_start(t[:], io["nCimT"].ap()[pt * 128 : (pt + 1) * 128, :])
                nci_t.append(t)
            xr_t = [xrp.tile([128, S], bf16, name=f"xr{pt}", tag=f"xr{pt}")
                    for pt in range(PTP)]
            xi_t = [xrp.tile([128, S], bf16, name=f"xi{pt}", tag=f"xi{pt}")
                    for pt in range(PTP)]
            for n in range(NM):
                ns = slice(n * 512, (n + 1) * 512)
                for pt in range(PTP):
                    mpl = p3.tile([128, 512], bf16, name="mpl", tag="mpl",
                                  bufs=4)
                    eng2 = A if (pt % 2 == 0) else SP
                    eng2.dma_start(mpl[:],
                                   io["mptab"].ap()[pt * 128 : (pt + 1) * 128, ns])
                    y2l = p3.tile([128, 1024], bf16, name="y2l", tag="y2l",
                                  bufs=4)
                    eng = SP if (pt % 2 == 0) else A
                    eng.dma_start(
                        y2l.rearrange("p (c n) -> p c n", c=2),
                        y2sp[pt].rearrange("p (c s) -> p c s", c=2)[:, :, ns],
                    )
                    mp1 = p3.tile([128, 512], bf16, name="mp1", tag="mp1")
                    mp2 = p3.tile([128, 512], bf16, name="mp2", tag="mp2")
                    V.tensor_scalar_mul(mp1[:], mpl[:], Gre_t[1][pt][:, 0:1])
                    V.tensor_scalar_mul(mp2[:], mpl[:], Gim_t[1][pt][:, 0:1])
                    yrc = p3.tile([128, 512], bf16, name="yrc", tag="yrc")
                    yic = p3.tile([128, 512], bf16, name="yic", tag="yic")
                    V.tensor_tensor(yrc[:], y2l[:, 0:512], mp1[:], op=OP.add)
                    G.tensor_tensor(yic[:], y2l[:, 512:1024], mp2[:], op=OP.add)
                    ua = p3.tile([128, 512], bf16, name="ua", tag="ua", bufs=1)
                    ub = p3.tile([128, 512], bf16, name="ub", tag="ub", bufs=1)
                    uc = p3.tile([128, 512], bf16, name="uc", tag="uc", bufs=1)
                    ud = p3.tile([128, 512], bf16, name="ud", tag="ud", bufs=1)
                    V.tensor_tensor(ua[:], ctab_t[pt][:, ns], yrc[:], op=OP.mult)
                    G.tensor_tensor(ub[:], stab_t[pt][:, ns], yic[:], op=OP.mult)
                    G.tensor_tensor(uc[:], stab_t[pt][:, ns], yrc[:], op=OP.mult)
                    V.tensor_tensor(ud[:], ctab_t[pt][:, ns], yic[:], op=OP.mult)
                    V.tensor_tensor(xr_t[pt][:, ns], ua[:], ub[:], op=OP.add)
                    G.tensor_tensor(xi_t[pt][:, ns], uc[:], ud[:], op=OP.subtract)
                for hb in range(HT):
                    mc_ = slice(hb * 128, (hb + 1) * 128)
                    op_ = ps3.tile([128, 512], f32, name="op", tag="op", bufs=8)
                    for k in range(PTP):
                        nc.tensor.matmul(
                            op_[:], cre_t[k][:, mc_],
                            xr_t[k][:, ns],
                            start=(k == 0), stop=False,
                        )
                    for k in range(PTP):
                        nc.tensor.matmul(
                            op_[:], nci_t[k][:, mc_],
                            xi_t[k][:, ns],
                            start=False, stop=(k == PTP - 1),
                        )
                    osb = p3.tile([128, 512], f32, name="osb", tag="osb", bufs=2)
                    A.copy(osb[:], op_[:])
                    SP.dma_start(io["outT"].ap()[mc_, ns], osb[:])

    est.close()


def build_program(cfg):
    import concourse.bacc as bacc
    import concourse.mybir as mybir
    import concourse.tile as tile

    f32 = mybir.dt.float32
    bf16 = mybir.dt.bfloat16
    S = cfg["L"] // NCORES

    nc = bacc.Bacc(
        "TRN2", target_bir_lowering=False, debug=False, num_devices=NCORES
    )
    io = {}
    ins = [
        ("uT", (H, S), bf16),
        ("BTre", (H, P), bf16), ("BTim", (H, P), bf16),
        ("CreT", (P, H), bf16), ("nCimT", (P, H), bf16),
        ("MT", (P, P), bf16),
        ("ctab", (P, S), bf16), ("stab", (P, S), bf16),
        ("mctab", (P, S), bf16), ("mstab", (P, S), bf16),
        ("mptab", (P, S), bf16),
        ("ptab", (P, _PT_NC), f32),
    ]
    for name, shape, dt_ in ins:
        io[name] = nc.dram_tensor(name, list(shape), dt_, kind="ExternalInput")
    io["outT"] = nc.dram_tensor("outT", [H, S], f32, kind="ExternalOutput")

    with tile.TileContext(nc) as tc:
        _emit(nc, tc, io, cfg)
    nc.compile()
    return nc


# ======================================================================
# host side
# ======================================================================

def make_tables(lam_re, lam_im, cfg):
    S = cfg["L"] // NCORES
    f32 = np.float32
    bf = ml_dtypes.bfloat16
    lam = lam_re.astype(np.float64) + 1j * lam_im.astype(np.float64)
    mag = np.abs(lam)
    th = np.angle(lam)
    k = np.arange(S)
    ang = np.outer(th, k)
    ctab = np.cos(ang)
    stab = np.sin(ang)
    with np.errstate(under="ignore"):
        mp = mag[:, None] ** k[None, :]
    tabs = dict(
        ctab=ctab.astype(bf), stab=stab.astype(bf),
        mctab=(mp * ctab).astype(bf), mstab=(mp * stab).astype(bf),
        mptab=mp.astype(bf),
    )
    coefre = np.zeros((NCORES, P, NCORES), f32)
    coefim = np.zeros((NCORES, P, NCORES), f32)
    for m in range(NCORES):
        for j in range(m):
            v = lam ** (S * (m - 1 - j))
            coefre[m, :, j] = np.real(v)
            coefim[m, :, j] = np.imag(v)
    ptab = np.zeros((NCORES, P, _PT_NC), f32)
    for m in range(NCORES):
        ptab[m, :, _PT_COEFRE : _PT_COEFRE + NCORES] = coefre[m]
        ptab[m, :, _PT_COEFIM : _PT_COEFIM + NCORES] = coefim[m]
        ptab[m, :, _PT_LPR] = np.real(lam)
        ptab[m, :, _PT_LPI] = np.imag(lam)
        ptab[m, :, _PT_COST1] = np.cos((S - 1) * th)
        ptab[m, :, _PT_SINT1] = np.sin((S - 1) * th)
        ptab[m, :, _PT_MVEC] = mag
        ptab[m, :, _PT_ZCOL] = 0.0 if m == 0 else 1.0
    return tabs, ptab


def make_in_maps(inputs, cfg):
    f32 = np.float32
    bf = ml_dtypes.bfloat16
    Lc = cfg["L"]
    S = Lc // NCORES
    u = np.ascontiguousarray(np.asarray(inputs["input_sequence"], f32)[:Lc])
    tabs, ptab = make_tables(
        np.asarray(inputs["Lambda_re"]), np.asarray(inputs["Lambda_im"]), cfg
    )
    M = (
        np.asarray(inputs["E"], np.float64)
        @ np.asarray(inputs["Delta"], np.float64).T
        @ np.asarray(inputs["F"], np.float64)
    )
    shared = dict(
        BTre=np.ascontiguousarray(np.asarray(inputs["B_re"], f32).T).astype(bf),
        BTim=np.ascontiguousarray(np.asarray(inputs["B_im"], f32).T).astype(bf),
        CreT=np.ascontiguousarray(np.asarray(inputs["C_re"], f32).T).astype(bf),
        nCimT=np.ascontiguousarray(-np.asarray(inputs["C_im"], f32).T).astype(bf),
        MT=np.ascontiguousarray(M.T).astype(bf),
        **tabs,
    )
    in_maps = []
    for m in range(NCORES):
        im = dict(shared)
        im["uT"] = np.ascontiguousarray(u[m * S : (m + 1) * S, :].T).astype(bf)
        im["ptab"] = np.ascontiguousarray(ptab[m])
        in_maps.append(im)
    return in_maps


def assemble_output(results, inputs, cfg):
    Lc = cfg["L"]
    S = Lc // NCORES
    out = np.empty((Lc, H), np.float32)
    for m in range(NCORES):
        out[m * S : (m + 1) * S, :] = results[m]["outT"].T
    # D*u term applied on host (elementwise on inputs; off the scan path)
    u = np.asarray(inputs["input_sequence"], np.float32)[:Lc]
    D = np.asarray(inputs["D"], np.float32)
    out += u * D
    out[0, :] = 0.0
    return out


def get_program(cfg_key="full"):
    if cfg_key not in _PROG_CACHE:
        _PROG_CACHE[cfg_key] = build_program(CFG_FULL)
    return _PROG_CACHE[cfg_key]


def run(inputs, trace=False, **kw):
    from concourse import bass_utils

    nc = get_program()
    in_maps = make_in_maps(inputs, CFG_FULL)
    res = bass_utils.run_bass_kernel_spmd(
        nc, in_maps, core_ids=list(range(NCORES)), trace=trace, **kw
    )
    return assemble_output(res.results, inputs, CFG_FULL), res


def kernel(**inputs):
    out, _ = run(inputs)
    return out
